# revision 1
# baseline (speedup 1.0000x reference)
"""KANLayer (in=128, out=128, num=5, k=3, batch=1024) on 8 trn2 NeuronCores.

Math: out[b,o] = sum_i mask*scale_base*silu(x[b,i])
              + sum_i mask*scale_sp*sum_j coef[(o,i),j]*B_j(x[b,i])
The reference grid is a uniform linspace broadcast to all rows, so the
Cox-de-Boor bases are cardinal cubic B-splines: B_j(x) = N3(v - j) with
v = (x - g0ext)/h.  N3 is evaluated in closed (truncated-power) form:
6*N3(v - j) = Delta^4[relu(v - n)^3] at n = j, so the whole basis bank
[128 i-lanes, 8 bases, batch] comes from one relu^3 over 12 taps plus
four shifted-slice subtracts (the 1/6 is folded into the spline weights
host-side).  The contraction over (i, j) and the silu base term are 9
accumulated 128x128x128 PE matmuls per core (bf16 in, f32 accumulate).

Valid for |x| <= g_ext_max (|x| <= 8.8 here); beyond that the Delta^4
cancellation noise grows cubically.  setup_inputs' randn never leaves
[-5, 5].

Sharding: batch 1024 -> 128 per core (independent; no collectives).

Execution: the Bass program is AOT-compiled once into a PJRT executable
(fast-dispatch, no per-call retrace) and dispatched on cores 0-7; falls
back to the stock run_bass_kernel_spmd path on any failure.
"""

import numpy as np

import concourse.bass as bass
import concourse.mybir as mybir
import concourse.tile as tile

AF = mybir.ActivationFunctionType
ALU = mybir.AluOpType
F32 = mybir.dt.float32
BF16 = mybir.dt.bfloat16

N_CORES = 8
BATCH = 1024
IN_DIM = 128
OUT_DIM = 128
NUM, KDEG = 5, 3
NB = NUM + KDEG          # 8 basis functions
NK = 1 + NB              # 9 matmul K-tiles (silu + 8 bases)
NT = NB + 4              # 12 truncated-power taps relu(v-n)^3, n = 0..11
BSH = BATCH // N_CORES   # 128 batch elems per core
SIZE = IN_DIM * OUT_DIM


def _bcast_mid(ap2d, n):
    """[128, F] AP -> [128, n, F] with zero-stride middle dim."""
    p, f = ap2d.shape
    return ap2d.rearrange("p (a b) -> p a b", a=1).broadcast_to([p, n, f])


def _flat(ap3d):
    """[128, a, b] AP -> [128, a*b]."""
    return ap3d.rearrange("p a b -> p (a b)")


MM_DT = BF16  # matmul operand dtype (weights, silu, basis bank)

# Live-tap window, set by prepare_inputs from the actual data range:
# tap n contributes only if max(v) > n, and relu is needed for tap n only
# if min(v) < n.  For randn x on this grid: v in [2.7, 8.3] -> L=9, N0=3.
TAP_L = NT
TAP_N0 = 0


def _emit_iter(nc, pool, psum, xs, wt, outT, inv_h, bias_v, split=False):
    """One full kernel pass: load, spline-basis bank, 9 matmuls, store.

    outT is a [OUT_DIM, BSH] dram AP (a per-iteration slice when unrolled
    pipelined).  split=False keeps the whole basis chain on the vector
    engine — same-engine deps are free and cross-engine hops cost ~1us,
    which is optimal when iterations serialize.  split=True interleaves
    vector and gpsimd per level for the double-buffered streaming case,
    where hops hide and the per-engine queue length is what matters.
    Every Delta level is one flat 2D op (in1 = same buffer shifted by BSH
    elements); 3D strided APs cost ~0.7us extra per op on DVE.
    """
    L = TAP_L  # L in [9, 12]
    eng2 = nc.gpsimd if split else nc.vector
    # double-buffered input loads: the next pass's DMA issues while this
    # pass computes, hiding the ~5us DMA latency (standard prefetch)
    X = pool.tile([128, BSH], F32, tag="X", bufs=2)
    nc.sync.dma_start(X[:], xs[:])
    WT = pool.tile([128, NK, OUT_DIM], MM_DT, tag="WT", bufs=2)
    nc.sync.dma_start(WT[:].rearrange("p a b -> p (a b)"), wt[:])

    S = pool.tile([128, BSH], MM_DT, tag="S")  # silu(x), matmul K-tile 0
    nc.scalar.activation(S[:], X[:], AF.Silu)

    # negated tap offsets -n (loop-constant; double-buffered so the next
    # iteration's iota never waits on this one's reader)
    ICW = pool.tile([128, L, BSH], F32, tag="ICW", bufs=2)
    nc.gpsimd.iota(
        ICW[:], pattern=[[-1, L], [0, BSH]], base=0, channel_multiplier=0,
        allow_small_or_imprecise_dtypes=True,
    )

    V = pool.tile([128, BSH], F32, tag="V")    # v = x/h - g0ext/h
    nc.vector.tensor_scalar(V[:], X[:], inv_h, bias_v, ALU.mult, ALU.add)
    D = pool.tile([128, L, BSH], F32, tag="D")      # v - n
    nc.vector.tensor_tensor(D[:], ICW[:], _bcast_mid(V[:], L), ALU.add)
    # relu on all taps (identity on the always-positive low taps)
    R = pool.tile([128, L, BSH], F32, tag="R")      # relu(v - n)
    nc.vector.tensor_scalar(_flat(R[:]), _flat(D[:]), 0.0, None, ALU.max)
    R2 = pool.tile([128, L, BSH], F32, tag="R2")
    nc.scalar.activation(_flat(R2[:]), _flat(R[:]), AF.Square)
    R3 = pool.tile([128, L + 1, BSH], F32, tag="R3")
    R3f = _flat(R3[:])
    nc.vector.tensor_tensor(R3f[:, : L * BSH], _flat(R2[:]), _flat(R[:]),
                            ALU.mult)
    eng2.memset(R3f[:, L * BSH :], 0.0)

    # BB[j] = Delta^4 R3 | j == 6 * B_j(v).  Rows >= L of each level are
    # identically zero (taps above the data range), kept as memset tail
    # rows so every level stays one op.
    D1 = pool.tile([128, L + 1, BSH], F32, tag="D1")
    D1f = _flat(D1[:])
    nc.gpsimd.tensor_tensor(D1f[:, : L * BSH], R3f[:, BSH:],
                            R3f[:, : L * BSH], ALU.subtract)
    nc.gpsimd.memset(D1f[:, L * BSH :], 0.0)
    D2 = pool.tile([128, L + 1, BSH], F32, tag="D2")
    D2f = _flat(D2[:])
    nc.vector.tensor_tensor(D2f[:, : L * BSH], D1f[:, BSH:],
                            D1f[:, : L * BSH], ALU.subtract)
    nc.vector.memset(D2f[:, L * BSH :], 0.0)
    D3 = pool.tile([128, NB + 1, BSH], F32, tag="D3")
    D3f = _flat(D3[:])
    eng2.tensor_tensor(D3f[:], D2f[:, BSH : (NB + 2) * BSH],
                       D2f[:, : (NB + 1) * BSH], ALU.subtract)
    BB = pool.tile([128, NB, BSH], MM_DT, tag="BB")
    nc.vector.tensor_tensor(_flat(BB[:]), D3f[:, BSH:], D3f[:, : NB * BSH],
                            ALU.subtract)

    # out^T[o,b] = sum_k WT[:,k,:]^T @ rhs_k, K = 9*128
    PS = psum.tile([OUT_DIM, BSH], F32, tag="PS", bufs=2)
    for k in range(NK):
        rhs = S[:] if k == 0 else BB[:, k - 1, :]
        nc.tensor.matmul(
            PS[:], WT[:, k, :], rhs, start=(k == 0), stop=(k == NK - 1)
        )
    # O double-buffered: the next pass's PSUM copy must not wait for this
    # pass's out-DMA (~5us latency) to release the staging tile
    O = pool.tile([OUT_DIM, BSH], F32, tag="O", bufs=2)
    nc.scalar.copy(O[:], PS[:])
    nc.sync.dma_start(outT[:, :], O[:])


def build_program(
    inv_h: float, bias_v: float, iters: int = 1, pipelined: bool = False
):
    """One SPMD NeuronCore program; per-core inputs differ only in data.

    iters > 1 unrolls the full kernel back-to-back inside one NEFF — used
    to measure per-iteration HW execution time without a profiler.
    pipelined=True double-buffers tiles and gives each iteration its own
    output slice (streaming steady state); False reuses single buffers,
    serializing iterations (per-pass latency).
    """
    nc = bass.Bass()
    xs = nc.declare_dram_parameter("xs", [IN_DIM, BSH], F32, isOutput=False)
    # weights pre-transposed host-side to [i, k*o] so the load is one
    # contiguous-per-partition DMA (the [k*i, o] layout needs a 1152-
    # descriptor gather, ~3us of DMA-queue time per pass)
    wt = nc.declare_dram_parameter(
        "wt", [128, NK * OUT_DIM], MM_DT, isOutput=False
    )
    n_out = iters if pipelined else 1
    outT = nc.declare_dram_parameter(
        "outT", [OUT_DIM, n_out * BSH], F32, isOutput=True
    )

    with tile.TileContext(nc) as tc:
        with (
            tc.tile_pool(name="pool", bufs=2 if pipelined else 1) as pool,
            tc.tile_pool(
                name="psum", bufs=2 if pipelined else 1,
                space=bass.MemorySpace.PSUM,
            ) as psum,
        ):
            for it in range(iters):
                o = outT[:, it * BSH : (it + 1) * BSH] if pipelined else outT[:]
                _emit_iter(nc, pool, psum, xs, wt, o, inv_h, bias_v,
                           split=pipelined)

    return nc


def _legalize_waits(nc):
    """Walrus codegen allows only one semaphore wait per compute/DMA
    instruction; move extra waits onto inserted same-engine NoOps."""
    for blk in nc.m.functions[0].blocks:
        out = []
        for ins in blk.instructions:
            si = ins.sync_info
            if si is not None and len(si.on_wait) > 1:
                waits = list(si.on_wait)
                for i, w in enumerate(waits[:-1]):
                    nop = mybir.InstNoOp(
                        name=f"{ins.name}-lw{i}", engine=ins.engine, ins=[], outs=[]
                    )
                    nop.sync_info = mybir.SyncInfo(on_wait=[w], on_update=[])
                    out.append(nop)
                ins.sync_info = mybir.SyncInfo(
                    on_wait=[waits[-1]], on_update=list(si.on_update)
                )
            out.append(ins)
        blk.instructions = out
    return nc


def prepare_inputs(x, grid, coef, scale_base, scale_sp, mask):
    global TAP_L, TAP_N0
    x = np.ascontiguousarray(x, dtype=np.float32)
    grid = np.asarray(grid, dtype=np.float32)
    coef = np.asarray(coef, dtype=np.float32)
    g = grid[0].astype(np.float64)
    h = (g[-1] - g[0]) / (len(g) - 1)
    g0ext = g[0] - KDEG * h
    inv_h = 1.0 / h
    bias_v = -g0ext * inv_h

    # live-tap window from the actual data range (v = x*inv_h + bias_v):
    # taps >= L are identically zero, taps < N0 never need the relu
    vmin = float(x.min()) * inv_h + bias_v
    vmax = float(x.max()) * inv_h + bias_v
    TAP_L = int(min(max(np.floor(vmax) + 1, 9), NT))
    TAP_N0 = int(max(min(np.floor(vmin) + 1, TAP_L), 0))

    import ml_dtypes

    sbm = (np.asarray(scale_base) * np.asarray(mask)).astype(np.float32)
    sspm = (np.asarray(scale_sp) * np.asarray(mask)).astype(np.float32)
    wt = np.empty((NK * 128, OUT_DIM), np.float32)
    wt[0:128] = sbm.reshape(OUT_DIM, IN_DIM).T
    # fold the 1/6 of the truncated-power form into the spline weights
    for j in range(NB):
        wt[(j + 1) * 128 : (j + 2) * 128] = (
            (sspm * coef[:, j] / 6.0).reshape(OUT_DIM, IN_DIM).T
        )
    # [k*i, o] -> [i, k*o] so each partition's weights are contiguous
    wt = np.ascontiguousarray(
        wt.reshape(NK, IN_DIM, OUT_DIM).transpose(1, 0, 2).reshape(
            IN_DIM, NK * OUT_DIM
        )
    ).astype(mybir.dt.np(MM_DT))
    xT = np.ascontiguousarray(x.T)  # [i, b]
    in_maps = [
        {
            "xs": np.ascontiguousarray(xT[:, c * BSH : (c + 1) * BSH]),
            "wt": wt,
        }
        for c in range(N_CORES)
    ]
    return in_maps, float(inv_h), float(bias_v)


class Runner:
    """AOT-compiled fast-dispatch executor for a Bass program on N cores.

    Compiles once (jit trace + NEFF build happen here, not per call);
    subsequent __call__s hit JAX's C++ fast path — per-call cost is the
    axon dispatch plus device execution only.
    """

    def __init__(self, nc, n_cores: int = N_CORES):
        import jax
        from jax.sharding import Mesh, NamedSharding, PartitionSpec

        from concourse import bass2jax
        from concourse.bass2jax import (
            _bass_exec_p,
            fast_dispatch_compile,
            install_neuronx_cc_hook,
        )

        try:
            from jax.experimental.shard_map import shard_map
        except ImportError:  # newer jax
            from jax import shard_map

        install_neuronx_cc_hook()
        self.jax = jax
        self.n_cores = n_cores
        part_name = nc.partition_id_tensor.name if nc.partition_id_tensor else None
        assert nc.dbg_addr is None

        in_names, in_shapes, out_names, out_avals = [], [], [], []
        for alloc in nc.m.functions[0].allocations:
            if not isinstance(alloc, mybir.MemoryLocationSet):
                continue
            name = alloc.memorylocations[0].name
            if alloc.kind == "ExternalInput":
                if name != part_name:
                    in_names.append(name)
                    in_shapes.append(
                        (tuple(alloc.tensor_shape), mybir.dt.np(alloc.dtype))
                    )
            elif alloc.kind == "ExternalOutput":
                out_names.append(name)
                out_avals.append(
                    jax.core.ShapedArray(
                        tuple(alloc.tensor_shape), mybir.dt.np(alloc.dtype)
                    )
                )
        self.in_names = in_names
        self.out_names = out_names
        self.out_avals = out_avals
        # The kernel writes every element of its outputs, so they are not
        # passed as (donated zero) operands — results are fresh buffers.
        all_in_names = list(in_names)
        if part_name is not None:
            all_in_names.append(part_name)

        def _body(*args):
            operands = list(args)
            if part_name is not None:
                operands.append(bass2jax.partition_id_tensor())
            outs = _bass_exec_p.bind(
                *operands,
                out_avals=tuple(out_avals),
                in_names=tuple(all_in_names),
                out_names=tuple(out_names),
                lowering_input_output_aliases=(),
                sim_require_finite=True,
                sim_require_nnan=True,
                nc=nc,
            )
            return tuple(outs)

        devices = jax.devices()[:n_cores]
        self.mesh = Mesh(np.asarray(devices), ("core",))
        self.sharding = NamedSharding(self.mesh, PartitionSpec("core"))
        in_specs = (PartitionSpec("core"),) * len(in_names)
        out_specs = (PartitionSpec("core"),) * len(out_names)
        jitted = jax.jit(
            shard_map(
                _body,
                mesh=self.mesh,
                in_specs=in_specs,
                out_specs=out_specs,
                check_rep=False,
            ),
            keep_unused=True,
        )

        def compile_fn():
            abstract = [
                jax.ShapeDtypeStruct((n_cores * s[0], *s[1:]), d)
                for (s, d) in in_shapes
            ]
            return jitted.lower(*abstract).compile()

        self.compiled = fast_dispatch_compile(compile_fn)

    def stage(self, in_maps):
        """Concat per-core inputs on axis 0 and put on device (committed)."""
        concat = [
            np.concatenate(
                [np.asarray(in_maps[c][nm]) for c in range(self.n_cores)], axis=0
            )
            for nm in self.in_names
        ]
        args = [self.jax.device_put(a, self.sharding) for a in concat]
        self.jax.block_until_ready(args)
        return args

    def __call__(self, args):
        return self.compiled(*args)

    def fetch_np(self, outs):
        """outs -> list of per-core np arrays for output 0."""
        arr = np.asarray(outs[0])
        s = self.out_avals[0].shape
        return arr.reshape(self.n_cores, *s)


def _assemble(per_core_outT):
    """per-core outT [OUT_DIM, BSH] -> full [BATCH, OUT_DIM]."""
    return np.ascontiguousarray(
        np.concatenate([o.T for o in per_core_outT], axis=0), dtype=np.float32
    )


def run(inputs: dict, trace: bool = False, **spmd_kwargs):
    """Stock-path execution (kept for debugging / fallback)."""
    from concourse.bass_utils import run_bass_kernel_spmd

    in_maps, inv_h, bias_v = prepare_inputs(**inputs)
    nc = _legalize_waits(build_program(inv_h, bias_v))
    res = run_bass_kernel_spmd(
        nc, in_maps, list(range(N_CORES)), trace=trace, **spmd_kwargs
    )
    out = _assemble([np.asarray(res.results[c]["outT"]) for c in range(N_CORES)])
    return out, res


def kernel(**inputs) -> np.ndarray:
    assert inputs["x"].shape == (BATCH, IN_DIM)
    in_maps, inv_h, bias_v = prepare_inputs(**inputs)
    nc = _legalize_waits(build_program(inv_h, bias_v))
    try:
        runner = Runner(nc)
        outs = runner(runner.stage(in_maps))
        return _assemble(list(runner.fetch_np(outs)))
    except Exception:
        from concourse.bass_utils import run_bass_kernel_spmd

        res = run_bass_kernel_spmd(nc, in_maps, list(range(N_CORES)))
        return _assemble(
            [np.asarray(res.results[c]["outT"]) for c in range(N_CORES)]
        )



# revision 5
# speedup vs baseline: 2.8600x; 2.8600x over previous
"""KANLayer (in=128, out=128, num=5, k=3, batch=1024) on 8 trn2 NeuronCores.

Math: out[b,o] = sum_i mask*scale_base*silu(x[b,i])
              + sum_i mask*scale_sp*sum_j coef[(o,i),j]*B_j(x[b,i])
The reference grid is a uniform linspace broadcast to all rows, so the
Cox-de-Boor bases are cardinal cubic B-splines, B_j(v) = Delta^4
relu(v-n)^3/6 at n=j with v = (x - g0ext)/h.  The Delta^4 is a fixed
linear map from truncated-power taps to bases, so it is folded into the
spline weights host-side: y_sp = sum_n W[s,n]*relu(v-n)^3 with
W = (coef/6) @ Delta4^T.  Three exact host-side rewrites then keep every
on-device tap value small enough for bf16 throughout:

  * taps with n >= vmax are identically zero -> dropped;
  * taps with n <= vmin satisfy relu(v-n)^3 = (v-n)^3 -> folded into a
    cubic polynomial;
  * live taps below the range midpoint use the mirror identity
    relu(v-n)^3 = (v-n)^3 + relu(n-v)^3, the cubic again folded into
    the polynomial.

The polynomial is rebased at the data midpoint cc, giving K-tiles
{silu(x), w, w^2, w^3, 6 truncated-power taps} with w = v - cc, every
tap |u| <= ~3 (cubes <= ~25, safe in bf16), and the constant term folded
into the output-copy bias on the scalar engine.  The device-side basis
work collapses from the baseline's eight 1152-wide f32 vector ops to
four 768-wide bf16 ops (2x/4x DVE modes) plus three 128-wide ones:
relu and cube commute (relu(u)^3 = relu(u^3)), so the chain is
shift -> square -> cube -> relu with the relu a 4x-mode tensor_scalar.
The contraction is 10 accumulated 128x128x128 bf16 PE matmuls per core.

Sharding: batch 1024 -> 128 per core (independent; no collectives).

Execution: the Bass program is AOT-compiled once into a PJRT executable
(fast-dispatch, no per-call retrace) and dispatched on cores 0-7; falls
back to the stock run_bass_kernel_spmd path on any failure.
"""

import numpy as np

import concourse.bass as bass
import concourse.mybir as mybir
import concourse.tile as tile

AF = mybir.ActivationFunctionType
ALU = mybir.AluOpType
F32 = mybir.dt.float32
BF16 = mybir.dt.bfloat16

N_CORES = 8
BATCH = 1024
IN_DIM = 128
OUT_DIM = 128
NUM, KDEG = 5, 3
NB = NUM + KDEG          # 8 basis functions
NT = NB + KDEG + 1       # 12 truncated-power taps
BSH = BATCH // N_CORES   # 128 batch elems per core
SIZE = IN_DIM * OUT_DIM

MM_DT = BF16  # matmul operand dtype


def _bcast_mid(ap2d, n):
    """[128, F] AP -> [128, n, F] with zero-stride middle dim."""
    p, f = ap2d.shape
    return ap2d.rearrange("p (a b) -> p a b", a=1).broadcast_to([p, n, f])


def _flat(ap3d):
    """[128, a, b] AP -> [128, a*b]."""
    return ap3d.rearrange("p a b -> p (a b)")


class Cfg:
    """Data-dependent program constants (live taps, split, poly center)."""

    def __init__(self, inv_h, bias_v, vmin, vmax):
        self.inv_h = float(inv_h)
        self.bias_v = float(bias_v)
        nlo = int(np.floor(vmin))          # taps <= nlo: always-on cubic
        nhi = int(np.ceil(vmax))           # taps >= nhi: identically zero
        self.cc = float((vmin + vmax) / 2)  # poly/tap rebase center
        self.msp = int(np.floor(self.cc))   # mirror split knot
        self.live = [n for n in range(max(nlo + 1, 0), min(nhi, NT))]
        self.nmir = sum(1 for n in self.live if n <= self.msp)
        self.ntap = len(self.live)
        self.ntiles = 4 + self.ntap        # silu, w, w^2, w^3, taps


def _emit_iter(nc, pool, psum, xs, wt, outT, ICW, BV, BC, cfg):
    """One full kernel pass: load, tap bank, 4+ntap matmuls, store."""
    ntap, nmir = cfg.ntap, cfg.nmir
    bvc = cfg.bias_v - cfg.cc
    # double-buffered input loads: the next pass's DMA issues while this
    # pass computes, hiding DMA latency (standard prefetch)
    X = pool.tile([128, BSH], F32, tag="X", bufs=2)
    nc.sync.dma_start(X[:], xs[:])
    WT = pool.tile([128, cfg.ntiles, OUT_DIM], MM_DT, tag="WT", bufs=2)
    nc.sync.dma_start(WT[:].rearrange("p a b -> p (a b)"), wt[:])

    S = pool.tile([128, BSH], MM_DT, tag="S")    # silu(x), K-tile 0
    nc.scalar.activation(S[:], X[:], AF.Silu)
    WQ = pool.tile([128, BSH], MM_DT, tag="WQ")  # w = v - cc, K-tile 1
    nc.vector.tensor_scalar(WQ[:], X[:], cfg.inv_h, bvc, ALU.mult, ALU.add)
    W2 = pool.tile([128, BSH], MM_DT, tag="W2")  # w^2, K-tile 2
    nc.scalar.activation(W2[:], X[:], AF.Square, bias=BC[:, 0:1],
                         scale=cfg.inv_h)

    # tap bank: u = (n-cc) - w for mirrored taps, w - (n-cc) for direct;
    # relu(u)^3 = relu(u^3) so the chain is sub, square, cube, then one
    # 4x-mode tensor_scalar max.  All bf16 (2x/4x DVE modes).
    DD = pool.tile([128, ntap, BSH], MM_DT, tag="DD")
    nc.vector.tensor_tensor(
        DD[:, :nmir, :], ICW[:, :nmir, :], _bcast_mid(WQ[:], nmir),
        ALU.subtract,
    )
    nc.vector.tensor_tensor(
        DD[:, nmir:, :], _bcast_mid(WQ[:], ntap - nmir), ICW[:, nmir:, :],
        ALU.subtract,
    )
    W3 = pool.tile([128, BSH], MM_DT, tag="W3")  # w^3, K-tile 3
    nc.vector.tensor_tensor(W3[:], W2[:], WQ[:], ALU.mult)
    U2 = pool.tile([128, ntap, BSH], MM_DT, tag="U2")
    nc.vector.tensor_tensor(_flat(U2[:]), _flat(DD[:]), _flat(DD[:]), ALU.mult)
    U3 = pool.tile([128, ntap, BSH], MM_DT, tag="U3")
    nc.vector.tensor_tensor(_flat(U3[:]), _flat(U2[:]), _flat(DD[:]), ALU.mult)
    R3 = pool.tile([128, ntap, BSH], MM_DT, tag="R3")
    nc.vector.tensor_scalar(_flat(R3[:]), _flat(U3[:]), 0.0, None, ALU.max)

    # out^T[o,b] = sum_k WT[:,k,:]^T @ rhs_k, K = ntiles*128
    PS = psum.tile([OUT_DIM, BSH], F32, tag="PS", bufs=2)
    rhss = [S[:], WQ[:], W2[:], W3[:]] + [R3[:, t, :] for t in range(ntap)]
    for k, rhs in enumerate(rhss):
        nc.tensor.matmul(
            PS[:], WT[:, k, :], rhs, start=(k == 0), stop=(k == len(rhss) - 1)
        )
    # PSUM -> SBUF copy folds in the polynomial constant term (per-o bias);
    # O double-buffered so the next pass's copy never waits on this DMA
    O = pool.tile([OUT_DIM, BSH], F32, tag="O", bufs=2)
    nc.scalar.activation(O[:], PS[:], AF.Identity, bias=BV[:, 0:1])
    nc.sync.dma_start(outT[:, :], O[:])


def build_program(cfg, iters: int = 1, pipelined: bool = False):
    """One SPMD NeuronCore program; per-core inputs differ only in data.

    iters > 1 unrolls the full kernel back-to-back inside one NEFF - used
    to measure per-iteration HW execution time without a profiler.
    pipelined=True double-buffers tiles and gives each iteration its own
    output slice (streaming steady state); False reuses single buffers,
    serializing iterations (per-pass latency).
    """
    nc = bass.Bass()
    xs = nc.declare_dram_parameter("xs", [IN_DIM, BSH], F32, isOutput=False)
    # weights pre-transposed host-side to [i, k*o] so the load is one
    # contiguous-per-partition DMA
    wt = nc.declare_dram_parameter(
        "wt", [128, cfg.ntiles * OUT_DIM], MM_DT, isOutput=False
    )
    icw = nc.declare_dram_parameter(
        "icw", [128, cfg.ntap * BSH], MM_DT, isOutput=False
    )
    bv = nc.declare_dram_parameter("bv", [OUT_DIM, 1], F32, isOutput=False)
    n_out = iters if pipelined else 1
    outT = nc.declare_dram_parameter(
        "outT", [OUT_DIM, n_out * BSH], F32, isOutput=True
    )

    with tile.TileContext(nc) as tc:
        with (
            tc.tile_pool(name="pool", bufs=2 if pipelined else 1) as pool,
            tc.tile_pool(
                name="psum", bufs=2 if pipelined else 1,
                space=bass.MemorySpace.PSUM,
            ) as psum,
        ):
            # loop-invariant constants, loaded once per NEFF execution
            ICW = pool.tile([128, cfg.ntap, BSH], MM_DT, tag="ICW", bufs=1)
            nc.sync.dma_start(_flat(ICW[:]), icw[:])
            BV = pool.tile([OUT_DIM, 1], F32, tag="BV", bufs=1)
            nc.sync.dma_start(BV[:], bv[:])
            BC = pool.tile([128, 1], F32, tag="BC", bufs=1)
            nc.gpsimd.memset(BC[:], cfg.bias_v - cfg.cc)
            for it in range(iters):
                o = outT[:, it * BSH : (it + 1) * BSH] if pipelined else outT[:]
                _emit_iter(nc, pool, psum, xs, wt, o, ICW, BV, BC, cfg)

    return nc


def _legalize_waits(nc):
    """Walrus codegen allows only one semaphore wait per compute/DMA
    instruction; move extra waits onto inserted same-engine NoOps."""
    for blk in nc.m.functions[0].blocks:
        out = []
        for ins in blk.instructions:
            si = ins.sync_info
            if si is not None and len(si.on_wait) > 1:
                waits = list(si.on_wait)
                for i, w in enumerate(waits[:-1]):
                    nop = mybir.InstNoOp(
                        name=f"{ins.name}-lw{i}", engine=ins.engine, ins=[], outs=[]
                    )
                    nop.sync_info = mybir.SyncInfo(on_wait=[w], on_update=[])
                    out.append(nop)
                ins.sync_info = mybir.SyncInfo(
                    on_wait=[waits[-1]], on_update=list(si.on_update)
                )
            out.append(ins)
        blk.instructions = out
    return nc


def prepare_inputs(x, grid, coef, scale_base, scale_sp, mask):
    x = np.ascontiguousarray(x, dtype=np.float32)
    grid = np.asarray(grid, dtype=np.float32)
    coef = np.asarray(coef, dtype=np.float64)
    g = grid[0].astype(np.float64)
    h = (g[-1] - g[0]) / (len(g) - 1)
    g0ext = g[0] - KDEG * h
    inv_h = 1.0 / h
    bias_v = -g0ext * inv_h

    vmin = float(x.min()) * inv_h + bias_v
    vmax = float(x.max()) * inv_h + bias_v
    cfg = Cfg(inv_h, bias_v, vmin, vmax)

    import ml_dtypes
    from math import comb

    bfq = lambda a: np.asarray(a, np.float32).astype(ml_dtypes.bfloat16)

    # fold Delta^4 (and the 1/6) into per-tap weights: W[s,n]
    W = np.zeros((SIZE, NT))
    for j in range(NB):
        for m in range(KDEG + 2):
            W[:, j + m] += coef[:, j] / 6.0 * ((-1) ** m) * comb(KDEG + 1, m)
    # cubic-polynomial fold of taps n <= msp, rebased at cc
    a = np.zeros((SIZE, 4))
    for n in range(0, cfg.msp + 1):
        t = cfg.cc - n
        a[:, 0] += W[:, n] * t**3
        a[:, 1] += W[:, n] * 3 * t**2
        a[:, 2] += W[:, n] * 3 * t
        a[:, 3] += W[:, n]

    sbm = np.asarray(scale_base, np.float64) * np.asarray(mask, np.float64)
    sspm = np.asarray(scale_sp, np.float64) * np.asarray(mask, np.float64)
    rows = [sbm, sspm * a[:, 1], sspm * a[:, 2], sspm * a[:, 3]]
    rows += [sspm * W[:, n] for n in cfg.live]
    wt = np.empty((cfg.ntiles * 128, OUT_DIM), np.float32)
    for k, r in enumerate(rows):
        wt[k * 128 : (k + 1) * 128] = r.reshape(OUT_DIM, IN_DIM).T
    # [k*i, o] -> [i, k*o] so each partition's weights are contiguous
    wt = np.ascontiguousarray(
        wt.reshape(cfg.ntiles, IN_DIM, OUT_DIM).transpose(1, 0, 2).reshape(
            IN_DIM, cfg.ntiles * OUT_DIM
        )
    ).astype(mybir.dt.np(MM_DT))

    # per-o output bias: constant poly term summed over i
    bv = np.ascontiguousarray(
        (sspm * a[:, 0]).reshape(OUT_DIM, IN_DIM).sum(axis=1)[:, None],
        dtype=np.float32,
    )
    # tap offsets n - cc, one [ntap, BSH] block replicated per partition
    offs = bfq([n - cfg.cc for n in cfg.live])
    icw = np.ascontiguousarray(
        np.broadcast_to(
            np.repeat(offs, BSH)[None, :], (128, cfg.ntap * BSH)
        )
    )

    xT = np.ascontiguousarray(x.T)  # [i, b]
    in_maps = [
        {
            "xs": np.ascontiguousarray(xT[:, c * BSH : (c + 1) * BSH]),
            "wt": wt,
            "icw": icw,
            "bv": bv,
        }
        for c in range(N_CORES)
    ]
    return in_maps, cfg


class Runner:
    """AOT-compiled fast-dispatch executor for a Bass program on N cores.

    Compiles once (jit trace + NEFF build happen here, not per call);
    subsequent __call__s hit JAX's C++ fast path - per-call cost is the
    axon dispatch plus device execution only.
    """

    def __init__(self, nc, n_cores: int = N_CORES):
        import jax
        from jax.sharding import Mesh, NamedSharding, PartitionSpec

        from concourse import bass2jax
        from concourse.bass2jax import (
            _bass_exec_p,
            fast_dispatch_compile,
            install_neuronx_cc_hook,
        )

        try:
            from jax.experimental.shard_map import shard_map
        except ImportError:  # newer jax
            from jax import shard_map

        install_neuronx_cc_hook()
        self.jax = jax
        self.n_cores = n_cores
        part_name = nc.partition_id_tensor.name if nc.partition_id_tensor else None
        assert nc.dbg_addr is None

        in_names, in_shapes, out_names, out_avals = [], [], [], []
        for alloc in nc.m.functions[0].allocations:
            if not isinstance(alloc, mybir.MemoryLocationSet):
                continue
            name = alloc.memorylocations[0].name
            if alloc.kind == "ExternalInput":
                if name != part_name:
                    in_names.append(name)
                    in_shapes.append(
                        (tuple(alloc.tensor_shape), mybir.dt.np(alloc.dtype))
                    )
            elif alloc.kind == "ExternalOutput":
                out_names.append(name)
                out_avals.append(
                    jax.core.ShapedArray(
                        tuple(alloc.tensor_shape), mybir.dt.np(alloc.dtype)
                    )
                )
        self.in_names = in_names
        self.out_names = out_names
        self.out_avals = out_avals
        # The kernel writes every element of its outputs, so they are not
        # passed as (donated zero) operands - results are fresh buffers.
        all_in_names = list(in_names)
        if part_name is not None:
            all_in_names.append(part_name)

        def _body(*args):
            operands = list(args)
            if part_name is not None:
                operands.append(bass2jax.partition_id_tensor())
            outs = _bass_exec_p.bind(
                *operands,
                out_avals=tuple(out_avals),
                in_names=tuple(all_in_names),
                out_names=tuple(out_names),
                lowering_input_output_aliases=(),
                sim_require_finite=True,
                sim_require_nnan=True,
                nc=nc,
            )
            return tuple(outs)

        devices = jax.devices()[:n_cores]
        self.mesh = Mesh(np.asarray(devices), ("core",))
        self.sharding = NamedSharding(self.mesh, PartitionSpec("core"))
        in_specs = (PartitionSpec("core"),) * len(in_names)
        out_specs = (PartitionSpec("core"),) * len(out_names)
        jitted = jax.jit(
            shard_map(
                _body,
                mesh=self.mesh,
                in_specs=in_specs,
                out_specs=out_specs,
                check_rep=False,
            ),
            keep_unused=True,
        )

        def compile_fn():
            abstract = [
                jax.ShapeDtypeStruct((n_cores * s[0], *s[1:]), d)
                for (s, d) in in_shapes
            ]
            return jitted.lower(*abstract).compile()

        self.compiled = fast_dispatch_compile(compile_fn)

    def stage(self, in_maps):
        """Concat per-core inputs on axis 0 and put on device (committed)."""
        concat = [
            np.concatenate(
                [np.asarray(in_maps[c][nm]) for c in range(self.n_cores)], axis=0
            )
            for nm in self.in_names
        ]
        args = [self.jax.device_put(a, self.sharding) for a in concat]
        self.jax.block_until_ready(args)
        return args

    def __call__(self, args):
        return self.compiled(*args)

    def fetch_np(self, outs):
        """outs -> list of per-core np arrays for output 0."""
        arr = np.asarray(outs[0])
        s = self.out_avals[0].shape
        return arr.reshape(self.n_cores, *s)


def _assemble(per_core_outT):
    """per-core outT [OUT_DIM, BSH] -> full [BATCH, OUT_DIM]."""
    return np.ascontiguousarray(
        np.concatenate([o.T for o in per_core_outT], axis=0), dtype=np.float32
    )


def run(inputs: dict, trace: bool = False, **spmd_kwargs):
    """Stock-path execution (kept for debugging / fallback)."""
    from concourse.bass_utils import run_bass_kernel_spmd

    in_maps, cfg = prepare_inputs(**inputs)
    nc = _legalize_waits(build_program(cfg))
    res = run_bass_kernel_spmd(
        nc, in_maps, list(range(N_CORES)), trace=trace, **spmd_kwargs
    )
    out = _assemble([np.asarray(res.results[c]["outT"]) for c in range(N_CORES)])
    return out, res


def kernel(**inputs) -> np.ndarray:
    assert inputs["x"].shape == (BATCH, IN_DIM)
    in_maps, cfg = prepare_inputs(**inputs)
    nc = _legalize_waits(build_program(cfg))
    try:
        runner = Runner(nc)
        outs = runner(runner.stage(in_maps))
        return _assemble(list(runner.fetch_np(outs)))
    except Exception:
        from concourse.bass_utils import run_bass_kernel_spmd

        res = run_bass_kernel_spmd(nc, in_maps, list(range(N_CORES)))
        return _assemble(
            [np.asarray(res.results[c]["outT"]) for c in range(N_CORES)]
        )


# revision 32
# speedup vs baseline: 4.3165x; 1.5093x over previous
"""KANLayer (in=128, out=128, num=5, k=3, batch=1024) on 8 trn2 NeuronCores.

Math: out[b,o] = sum_i mask*scale_base*silu(x[b,i])
              + sum_i mask*scale_sp*sum_j coef[(o,i),j]*B_j(x[b,i])
The reference grid is a uniform linspace broadcast to all rows, so the
Cox-de-Boor bases are cardinal cubic B-splines, B_j(v) = Delta^4
relu(v-n)^3/6 at n=j with v = (x - g0ext)/h.  The Delta^4 is a fixed
linear map from truncated-power taps to bases, so it is folded into the
spline weights host-side: y_sp = sum_n W[s,n]*relu(v-n)^3 with
W = (coef/6) @ Delta4^T.  Three exact host-side rewrites then keep every
on-device lane value small enough for bf16 throughout:

  * taps with knot >= max(v) are identically zero -> dropped;
  * taps with knot <= min(v) satisfy relu(v-n)^3 = (v-n)^3 -> folded
    into a cubic polynomial;
  * live taps below the range midpoint use the mirror identity
    relu(v-n)^3 = (v-n)^3 + relu(n-v)^3, the cubic again folded into
    the polynomial.

Everything is evaluated in x-space (u = x - knot_x, the 1/h^k scales
folded into the weights).  The device-side work per pass is one bf16
lane bank [128, 7, 128] = {6 live knots, 1 poly-center lane} built by
four DVE ops (subtract; cube via square+multiply with the square on the
scalar engine; relu as min over mirrored lanes + max over direct lanes,
with the mirror sign folded into the weights since relu(knot-x)^3 =
-min(u^3, 0) and relu/cube commute), plus silu on the scalar engine.
The poly-center lane's powers in DD/U2/U3 are the polynomial features
w, w^2, w^3 for free, and the constant term rides the PSUM->SBUF output
copy as a per-partition bias.  The contraction is 10 accumulated
128x128x128 bf16 PE matmuls per core against a weight bank that stays
resident in SBUF.  All elementwise ops run in bf16 (2x/4x DVE modes).

Sharding: batch 1024 -> 128 per core (independent; no collectives).

Execution: the Bass program is AOT-compiled once into a PJRT executable
(fast-dispatch, no per-call retrace) and dispatched on cores 0-7; falls
back to the stock run_bass_kernel_spmd path on any failure.
"""

import numpy as np

import concourse.bass as bass
import concourse.mybir as mybir
import concourse.tile as tile

AF = mybir.ActivationFunctionType
ALU = mybir.AluOpType
F32 = mybir.dt.float32
BF16 = mybir.dt.bfloat16

N_CORES = 8
BATCH = 1024
IN_DIM = 128
OUT_DIM = 128
NUM, KDEG = 5, 3
NB = NUM + KDEG          # 8 basis functions
NT = NB + KDEG + 1       # 12 truncated-power taps
BSH = BATCH // N_CORES   # 128 batch elems per core
SIZE = IN_DIM * OUT_DIM

MM_DT = BF16  # matmul operand dtype


def _bcast_mid(ap2d, n):
    """[128, F] AP -> [128, n, F] with zero-stride middle dim."""
    p, f = ap2d.shape
    return ap2d.rearrange("p (a b) -> p a b", a=1).broadcast_to([p, n, f])


def _flat(ap3d):
    """[128, a, b] AP -> [128, a*b]."""
    return ap3d.rearrange("p a b -> p (a b)")


class Cfg:
    """Data-dependent program constants (live taps, split, poly center)."""

    def __init__(self, inv_h, bias_v, vmin, vmax):
        self.inv_h = float(inv_h)
        self.bias_v = float(bias_v)
        nlo = int(np.floor(vmin))          # taps <= nlo: always-on cubic
        nhi = int(np.ceil(vmax))           # taps >= nhi: identically zero
        self.cc = float((vmin + vmax) / 2)  # poly/tap rebase center
        self.msp = int(np.floor(self.cc))   # mirror split knot
        self.live = [n for n in range(max(nlo + 1, 0), min(nhi, NT))]
        self.nmir = sum(1 for n in self.live if n <= self.msp)
        self.ntap = len(self.live)
        self.nlanes = self.ntap + 1        # taps + the w-lane (t=0)
        self.ntiles = 4 + self.ntap        # silu, w, w^2, w^3, taps
        self.g0ext = 0.0                   # extended-grid origin (x-space)
        self.h = 1.0 / self.inv_h
        # engine assignment knobs (chosen via timeline-sim search)
        # (GPSIMD cannot access PSUM, so o_eng is scalar or vector only)
        self.o_eng = "scalar"  # PSUM->SBUF output copy engine
        self.u2_act = True     # lane square on Act engine (else DVE)
        self.nbufs = 3         # passes in flight (tile buffer depth)
        self.out_pool = False  # issue the output DMA from the Pool queue
        self.skew = True       # emit pass k's output stage after pass k+1


def _emit_out(nc, pool, PS, outT, BV, cfg):
    """Output stage: PSUM -> SBUF copy (+ poly-constant bias), then DMA."""
    O = pool.tile([OUT_DIM, BSH], F32, tag="O", bufs=cfg.nbufs)
    if cfg.o_eng == "vector":
        nc.vector.tensor_scalar(O[:], PS[:], BV[:, 0:1], None, ALU.add)
    elif cfg.o_eng == "gpsimd":
        nc.gpsimd.tensor_scalar(O[:], PS[:], BV[:, 0:1], None, ALU.add)
    else:
        nc.scalar.activation(O[:], PS[:], AF.Identity, bias=BV[:, 0:1])
    (nc.gpsimd if cfg.out_pool else nc.sync).dma_start(outT[:, :], O[:])


def _emit_iter(nc, pool, psum, xs, WT, ICW, cfg):
    """One pass's compute: load, lane bank, 4+ntap matmuls -> PSUM tile."""
    ntap, nmir = cfg.ntap, cfg.nmir
    ib = cfg.nbufs  # intermediate-tile buffering (passes in flight)
    # multi-buffered input load (x arrives pre-cast to bf16): later
    # passes' DMAs issue while this pass computes (standard prefetch)
    XB = pool.tile([128, BSH], MM_DT, tag="XB", bufs=ib)
    nc.sync.dma_start(XB[:], xs[:])

    S = pool.tile([128, BSH], MM_DT, tag="S", bufs=ib)  # silu(x), K-tile 0
    nc.scalar.activation(S[:], XB[:], AF.Silu)

    # lane bank in x-space: u = x - knot_x per live knot, plus a final
    # lane at the poly center whose powers are the poly features (the
    # 1/h^k scalings are folded into the weights host-side).  relu(u)^3 =
    # relu(u^3), and the mirrored (below-split) knots need relu(knot-x)^3
    # = -min(u^3, 0), so the sign fold goes into their weights and the
    # relu stage is one min over mirrored lanes + one max over direct
    # lanes (4x-mode tensor_scalar).  All bf16 (2x/4x DVE modes).
    nl = cfg.nlanes
    DD = pool.tile([128, nl, BSH], MM_DT, tag="DD", bufs=ib)
    nc.vector.tensor_tensor(
        DD[:], _bcast_mid(XB[:], nl), ICW[:], ALU.subtract
    )
    U2 = pool.tile([128, nl, BSH], MM_DT, tag="U2", bufs=ib)
    if cfg.u2_act:
        nc.scalar.activation(_flat(U2[:]), _flat(DD[:]), AF.Square)
    else:
        nc.vector.tensor_tensor(_flat(U2[:]), _flat(DD[:]), _flat(DD[:]),
                                ALU.mult)
    U3 = pool.tile([128, nl, BSH], MM_DT, tag="U3", bufs=ib)
    nc.vector.tensor_tensor(_flat(U3[:]), _flat(U2[:]), _flat(DD[:]), ALU.mult)
    R3 = pool.tile([128, ntap, BSH], MM_DT, tag="R3", bufs=ib)
    nc.vector.tensor_scalar(
        _flat(R3[:])[:, : nmir * BSH], _flat(U3[:])[:, : nmir * BSH],
        0.0, None, ALU.min,
    )
    nc.vector.tensor_scalar(
        _flat(R3[:])[:, nmir * BSH :],
        _flat(U3[:])[:, nmir * BSH : ntap * BSH], 0.0, None, ALU.max,
    )

    # out^T[o,b] = sum_k WT[:,k,:]^T @ rhs_k, K = ntiles*128
    PS = psum.tile([OUT_DIM, BSH], F32, tag="PS",
                   bufs=cfg.nbufs + (1 if cfg.skew else 0))
    rhss = [S[:], DD[:, ntap, :], U2[:, ntap, :], U3[:, ntap, :]]
    rhss += [R3[:, t, :] for t in range(ntap)]
    for k, rhs in enumerate(rhss):
        nc.tensor.matmul(
            PS[:], WT[:, k, :], rhs, start=(k == 0), stop=(k == len(rhss) - 1)
        )
    return PS


def build_program(
    cfg, iters: int = 1, pipelined: bool = False, loop_n: int = 1
):
    """One SPMD NeuronCore program; per-core inputs differ only in data.

    iters > 1 unrolls the full kernel back-to-back inside one NEFF, and
    loop_n > 1 wraps the unrolled body in a hardware For_i loop (total
    passes = iters * loop_n) - used to measure per-iteration HW execution
    time without a profiler while keeping the NEFF small.

    Successive passes write a small ring of output slices (a real stream
    writes each batch's result to a distinct buffer; reusing one address
    would add an artificial DRAM write-after-write serialization to the
    measurement).  Slice 0 always holds a complete pass result.
    """
    del pipelined  # legacy knob, superseded by the output ring
    nc = bass.Bass()
    xs = nc.declare_dram_parameter("xs", [IN_DIM, BSH], MM_DT, isOutput=False)
    # weights pre-transposed host-side to [i, k*o] so the load is one
    # contiguous-per-partition DMA
    wt = nc.declare_dram_parameter(
        "wt", [128, cfg.ntiles * OUT_DIM], MM_DT, isOutput=False
    )
    icw = nc.declare_dram_parameter(
        "icw", [128, cfg.nlanes * BSH], MM_DT, isOutput=False
    )
    bv = nc.declare_dram_parameter("bv", [OUT_DIM, 1], F32, isOutput=False)
    ring = min(iters, 8)
    outT = nc.declare_dram_parameter(
        "outT", [OUT_DIM, ring * BSH], F32, isOutput=True
    )

    with tile.TileContext(nc) as tc:
        with (
            tc.tile_pool(name="pool", bufs=1) as pool,
            tc.tile_pool(
                name="psum", bufs=1, space=bass.MemorySpace.PSUM,
            ) as psum,
        ):
            # loop-invariant constants, loaded once per NEFF execution:
            # tap offsets, output bias, w-shift, and the weight bank
            # (weights are pass-invariant, so they stay resident in SBUF)
            ICW = pool.tile([128, cfg.nlanes, BSH], MM_DT, tag="ICW", bufs=1)
            nc.sync.dma_start(_flat(ICW[:]), icw[:])
            BV = pool.tile([OUT_DIM, 1], F32, tag="BV", bufs=1)
            nc.sync.dma_start(BV[:], bv[:])
            WT = pool.tile([128, cfg.ntiles, OUT_DIM], MM_DT, tag="WT", bufs=1)
            nc.sync.dma_start(WT[:].rearrange("p a b -> p (a b)"), wt[:])

            def body():
                pending = None  # (PS, out-slice) awaiting its output stage
                for it in range(iters):
                    r = it % ring
                    o = outT[:, r * BSH : (r + 1) * BSH]
                    PS = _emit_iter(nc, pool, psum, xs, WT, ICW, cfg)
                    if not cfg.skew:
                        _emit_out(nc, pool, PS, o, BV, cfg)
                    else:
                        if pending is not None:
                            _emit_out(nc, pool, pending[0], pending[1], BV,
                                      cfg)
                        pending = (PS, o)
                if pending is not None:
                    _emit_out(nc, pool, pending[0], pending[1], BV, cfg)

            if loop_n > 1:
                with tc.For_i(0, loop_n, 1):
                    body()
            else:
                body()

    return nc


def _legalize_waits(nc):
    """Walrus codegen allows only one semaphore wait per compute/DMA
    instruction; move extra waits onto inserted same-engine NoOps."""
    for blk in nc.m.functions[0].blocks:
        out = []
        for ins in blk.instructions:
            si = ins.sync_info
            if si is not None and len(si.on_wait) > 1:
                waits = list(si.on_wait)
                for i, w in enumerate(waits[:-1]):
                    nop = mybir.InstNoOp(
                        name=f"{ins.name}-lw{i}", engine=ins.engine, ins=[], outs=[]
                    )
                    nop.sync_info = mybir.SyncInfo(on_wait=[w], on_update=[])
                    out.append(nop)
                ins.sync_info = mybir.SyncInfo(
                    on_wait=[waits[-1]], on_update=list(si.on_update)
                )
            out.append(ins)
        blk.instructions = out
    return nc


def prepare_inputs(x, grid, coef, scale_base, scale_sp, mask):
    x = np.ascontiguousarray(x, dtype=np.float32)
    grid = np.asarray(grid, dtype=np.float32)
    coef = np.asarray(coef, dtype=np.float64)
    g = grid[0].astype(np.float64)
    h = (g[-1] - g[0]) / (len(g) - 1)
    g0ext = g[0] - KDEG * h
    inv_h = 1.0 / h
    bias_v = -g0ext * inv_h

    vmin = float(x.min()) * inv_h + bias_v
    vmax = float(x.max()) * inv_h + bias_v
    cfg = Cfg(inv_h, bias_v, vmin, vmax)

    import ml_dtypes
    from math import comb

    bfq = lambda a: np.asarray(a, np.float32).astype(ml_dtypes.bfloat16)

    # fold Delta^4 (and the 1/6) into per-tap weights: W[s,n]
    W = np.zeros((SIZE, NT))
    for j in range(NB):
        for m in range(KDEG + 2):
            W[:, j + m] += coef[:, j] / 6.0 * ((-1) ** m) * comb(KDEG + 1, m)
    # cubic-polynomial fold of taps n <= msp, rebased at cc
    a = np.zeros((SIZE, 4))
    for n in range(0, cfg.msp + 1):
        t = cfg.cc - n
        a[:, 0] += W[:, n] * t**3
        a[:, 1] += W[:, n] * 3 * t**2
        a[:, 2] += W[:, n] * 3 * t
        a[:, 3] += W[:, n]

    sbm = np.asarray(scale_base, np.float64) * np.asarray(mask, np.float64)
    sspm = np.asarray(scale_sp, np.float64) * np.asarray(mask, np.float64)
    # 1/h^k folds for the x-space lane bank; mirrored knots get the
    # relu(knot-x)^3 = -min(u^3,0) sign fold
    rows = [sbm, sspm * a[:, 1] * inv_h, sspm * a[:, 2] * inv_h**2,
            sspm * a[:, 3] * inv_h**3]
    rows += [sspm * W[:, n] * inv_h**3 * (-1.0 if n <= cfg.msp else 1.0)
             for n in cfg.live]
    wt = np.empty((cfg.ntiles * 128, OUT_DIM), np.float32)
    for k, r in enumerate(rows):
        wt[k * 128 : (k + 1) * 128] = r.reshape(OUT_DIM, IN_DIM).T
    # [k*i, o] -> [i, k*o] so each partition's weights are contiguous
    wt = np.ascontiguousarray(
        wt.reshape(cfg.ntiles, IN_DIM, OUT_DIM).transpose(1, 0, 2).reshape(
            IN_DIM, cfg.ntiles * OUT_DIM
        )
    ).astype(mybir.dt.np(MM_DT))

    # per-o output bias: constant poly term summed over i
    bv = np.ascontiguousarray(
        (sspm * a[:, 0]).reshape(OUT_DIM, IN_DIM).sum(axis=1)[:, None],
        dtype=np.float32,
    )
    # lane offsets: knot x-positions, then the poly-center lane
    offs = bfq([g0ext + n * h for n in cfg.live] + [g0ext + cfg.cc * h])
    icw = np.ascontiguousarray(
        np.broadcast_to(
            np.repeat(offs, BSH)[None, :], (128, cfg.nlanes * BSH)
        )
    )

    xT = np.ascontiguousarray(x.T).astype(mybir.dt.np(MM_DT))  # [i, b] bf16
    in_maps = [
        {
            "xs": np.ascontiguousarray(xT[:, c * BSH : (c + 1) * BSH]),
            "wt": wt,
            "icw": icw,
            "bv": bv,
        }
        for c in range(N_CORES)
    ]
    return in_maps, cfg


class Runner:
    """AOT-compiled fast-dispatch executor for a Bass program on N cores.

    Compiles once (jit trace + NEFF build happen here, not per call);
    subsequent __call__s hit JAX's C++ fast path - per-call cost is the
    axon dispatch plus device execution only.
    """

    def __init__(self, nc, n_cores: int = N_CORES):
        import jax
        from jax.sharding import Mesh, NamedSharding, PartitionSpec

        from concourse import bass2jax
        from concourse.bass2jax import (
            _bass_exec_p,
            fast_dispatch_compile,
            install_neuronx_cc_hook,
        )

        try:
            from jax.experimental.shard_map import shard_map
        except ImportError:  # newer jax
            from jax import shard_map

        install_neuronx_cc_hook()
        self.jax = jax
        self.n_cores = n_cores
        part_name = nc.partition_id_tensor.name if nc.partition_id_tensor else None
        assert nc.dbg_addr is None

        in_names, in_shapes, out_names, out_avals = [], [], [], []
        for alloc in nc.m.functions[0].allocations:
            if not isinstance(alloc, mybir.MemoryLocationSet):
                continue
            name = alloc.memorylocations[0].name
            if alloc.kind == "ExternalInput":
                if name != part_name:
                    in_names.append(name)
                    in_shapes.append(
                        (tuple(alloc.tensor_shape), mybir.dt.np(alloc.dtype))
                    )
            elif alloc.kind == "ExternalOutput":
                out_names.append(name)
                out_avals.append(
                    jax.core.ShapedArray(
                        tuple(alloc.tensor_shape), mybir.dt.np(alloc.dtype)
                    )
                )
        self.in_names = in_names
        self.out_names = out_names
        self.out_avals = out_avals
        # The kernel writes every element of its outputs, so they are not
        # passed as (donated zero) operands - results are fresh buffers.
        all_in_names = list(in_names)
        if part_name is not None:
            all_in_names.append(part_name)

        def _body(*args):
            operands = list(args)
            if part_name is not None:
                operands.append(bass2jax.partition_id_tensor())
            outs = _bass_exec_p.bind(
                *operands,
                out_avals=tuple(out_avals),
                in_names=tuple(all_in_names),
                out_names=tuple(out_names),
                lowering_input_output_aliases=(),
                sim_require_finite=True,
                sim_require_nnan=True,
                nc=nc,
            )
            return tuple(outs)

        devices = jax.devices()[:n_cores]
        self.mesh = Mesh(np.asarray(devices), ("core",))
        self.sharding = NamedSharding(self.mesh, PartitionSpec("core"))
        in_specs = (PartitionSpec("core"),) * len(in_names)
        out_specs = (PartitionSpec("core"),) * len(out_names)
        jitted = jax.jit(
            shard_map(
                _body,
                mesh=self.mesh,
                in_specs=in_specs,
                out_specs=out_specs,
                check_rep=False,
            ),
            keep_unused=True,
        )

        def compile_fn():
            abstract = [
                jax.ShapeDtypeStruct((n_cores * s[0], *s[1:]), d)
                for (s, d) in in_shapes
            ]
            return jitted.lower(*abstract).compile()

        self.compiled = fast_dispatch_compile(compile_fn)

    def stage(self, in_maps):
        """Concat per-core inputs on axis 0 and put on device (committed)."""
        concat = [
            np.concatenate(
                [np.asarray(in_maps[c][nm]) for c in range(self.n_cores)], axis=0
            )
            for nm in self.in_names
        ]
        args = [self.jax.device_put(a, self.sharding) for a in concat]
        self.jax.block_until_ready(args)
        return args

    def __call__(self, args):
        return self.compiled(*args)

    def fetch_np(self, outs):
        """outs -> list of per-core np arrays for output 0."""
        arr = np.asarray(outs[0])
        s = self.out_avals[0].shape
        return arr.reshape(self.n_cores, *s)


def _assemble(per_core_outT):
    """per-core outT [OUT_DIM, BSH] -> full [BATCH, OUT_DIM]."""
    return np.ascontiguousarray(
        np.concatenate([o.T for o in per_core_outT], axis=0), dtype=np.float32
    )


def run(inputs: dict, trace: bool = False, **spmd_kwargs):
    """Stock-path execution (kept for debugging / fallback)."""
    from concourse.bass_utils import run_bass_kernel_spmd

    in_maps, cfg = prepare_inputs(**inputs)
    nc = _legalize_waits(build_program(cfg))
    res = run_bass_kernel_spmd(
        nc, in_maps, list(range(N_CORES)), trace=trace, **spmd_kwargs
    )
    out = _assemble([np.asarray(res.results[c]["outT"]) for c in range(N_CORES)])
    return out, res


def kernel(**inputs) -> np.ndarray:
    assert inputs["x"].shape == (BATCH, IN_DIM)
    in_maps, cfg = prepare_inputs(**inputs)
    nc = _legalize_waits(build_program(cfg))
    try:
        runner = Runner(nc)
        outs = runner(runner.stage(in_maps))
        return _assemble(list(runner.fetch_np(outs)))
    except Exception:
        from concourse.bass_utils import run_bass_kernel_spmd

        res = run_bass_kernel_spmd(nc, in_maps, list(range(N_CORES)))
        return _assemble(
            [np.asarray(res.results[c]["outT"]) for c in range(N_CORES)]
        )


# revision 33
# speedup vs baseline: 6.4298x; 1.4896x over previous
"""KANLayer (in=128, out=128, num=5, k=3, batch=1024) on 8 trn2 NeuronCores.

Math: out[b,o] = sum_i mask*scale_base*silu(x[b,i])
              + sum_i mask*scale_sp*sum_j coef[(o,i),j]*B_j(x[b,i])
The reference grid is a uniform linspace broadcast to all rows, so the
Cox-de-Boor bases are cardinal cubic B-splines, B_j(v) = Delta^4
relu(v-n)^3/6 at n=j with v = (x - g0ext)/h.  The Delta^4 is a fixed
linear map from truncated-power taps to bases, so it is folded into the
spline weights host-side: y_sp = sum_n W[s,n]*relu(v-n)^3 with
W = (coef/6) @ Delta4^T.  Three exact host-side rewrites then keep every
on-device lane value small enough for bf16 throughout:

  * taps with knot >= max(v) are identically zero -> dropped;
  * taps with knot <= min(v) satisfy relu(v-n)^3 = (v-n)^3 -> folded
    into a cubic polynomial;
  * live taps below the range midpoint use the mirror identity
    relu(v-n)^3 = (v-n)^3 + relu(n-v)^3, the cubic again folded into
    the polynomial.

Everything is evaluated in x-space (u = x - knot_x, the 1/h^k scales
folded into the weights).  The device-side work per pass is one bf16
lane bank [128, 7, 128] = {6 live knots, 1 poly-center lane} built by
four DVE ops (subtract; cube via square+multiply with the square on the
scalar engine; relu as min over mirrored lanes + max over direct lanes,
with the mirror sign folded into the weights since relu(knot-x)^3 =
-min(u^3, 0) and relu/cube commute), plus silu on the scalar engine.
The poly-center lane's powers in DD/U2/U3 are the polynomial features
w, w^2, w^3 for free, and the constant term rides the PSUM->SBUF output
copy as a per-partition bias.  The contraction is 10 accumulated
128x128x128 bf16 PE matmuls per core against a weight bank that stays
resident in SBUF.  All elementwise ops run in bf16 (2x/4x DVE modes).

Sharding: batch 1024 -> 128 per core (independent; no collectives).

Execution: the Bass program is AOT-compiled once into a PJRT executable
(fast-dispatch, no per-call retrace) and dispatched on cores 0-7; falls
back to the stock run_bass_kernel_spmd path on any failure.
"""

import numpy as np

import concourse.bass as bass
import concourse.mybir as mybir
import concourse.tile as tile

AF = mybir.ActivationFunctionType
ALU = mybir.AluOpType
F32 = mybir.dt.float32
BF16 = mybir.dt.bfloat16

N_CORES = 8
BATCH = 1024
IN_DIM = 128
OUT_DIM = 128
NUM, KDEG = 5, 3
NB = NUM + KDEG          # 8 basis functions
NT = NB + KDEG + 1       # 12 truncated-power taps
BSH = BATCH // N_CORES   # 128 batch elems per core
SIZE = IN_DIM * OUT_DIM

MM_DT = BF16  # matmul operand dtype


def _bcast_mid(ap2d, n):
    """[128, F] AP -> [128, n, F] with zero-stride middle dim."""
    p, f = ap2d.shape
    return ap2d.rearrange("p (a b) -> p a b", a=1).broadcast_to([p, n, f])


def _flat(ap3d):
    """[128, a, b] AP -> [128, a*b]."""
    return ap3d.rearrange("p a b -> p (a b)")


class Cfg:
    """Data-dependent program constants (live taps, split, poly center)."""

    def __init__(self, inv_h, bias_v, vmin, vmax):
        self.inv_h = float(inv_h)
        self.bias_v = float(bias_v)
        nlo = int(np.floor(vmin))          # taps <= nlo: always-on cubic
        nhi = int(np.ceil(vmax))           # taps >= nhi: identically zero
        self.cc = float((vmin + vmax) / 2)  # poly/tap rebase center
        self.msp = int(np.floor(self.cc))   # mirror split knot
        self.live = [n for n in range(max(nlo + 1, 0), min(nhi, NT))]
        self.nmir = sum(1 for n in self.live if n <= self.msp)
        self.ntap = len(self.live)
        self.nlanes = self.ntap + 1        # taps + the w-lane (t=0)
        self.ntiles = 4 + self.ntap        # silu, w, w^2, w^3, taps
        self.g0ext = 0.0                   # extended-grid origin (x-space)
        self.h = 1.0 / self.inv_h
        # engine assignment knobs (chosen by timeline-sim + on-device sweep)
        # (GPSIMD cannot access PSUM, so o_eng is scalar or vector only)
        self.o_eng = "vector"  # PSUM->SBUF output copy engine
        self.u2_act = True     # lane square on Act engine (else DVE)
        self.nbufs = 3         # passes in flight (tile buffer depth)
        self.out_pool = False  # issue the output DMA from the Pool queue
        self.skew = True       # emit pass k's output stage after pass k+1


def _emit_out(nc, pool, PS, outT, BV, cfg):
    """Output stage: PSUM -> SBUF copy (+ poly-constant bias), then DMA."""
    O = pool.tile([OUT_DIM, BSH], F32, tag="O", bufs=cfg.nbufs)
    if cfg.o_eng == "vector":
        nc.vector.tensor_scalar(O[:], PS[:], BV[:, 0:1], None, ALU.add)
    elif cfg.o_eng == "gpsimd":
        nc.gpsimd.tensor_scalar(O[:], PS[:], BV[:, 0:1], None, ALU.add)
    else:
        nc.scalar.activation(O[:], PS[:], AF.Identity, bias=BV[:, 0:1])
    (nc.gpsimd if cfg.out_pool else nc.sync).dma_start(outT[:, :], O[:])


def _emit_iter(nc, pool, psum, xs, WT, ICW, cfg):
    """One pass's compute: load, lane bank, 4+ntap matmuls -> PSUM tile."""
    ntap, nmir = cfg.ntap, cfg.nmir
    ib = cfg.nbufs  # intermediate-tile buffering (passes in flight)
    # multi-buffered input load (x arrives pre-cast to bf16): later
    # passes' DMAs issue while this pass computes (standard prefetch)
    XB = pool.tile([128, BSH], MM_DT, tag="XB", bufs=ib)
    nc.sync.dma_start(XB[:], xs[:])

    S = pool.tile([128, BSH], MM_DT, tag="S", bufs=ib)  # silu(x), K-tile 0
    nc.scalar.activation(S[:], XB[:], AF.Silu)

    # lane bank in x-space: u = x - knot_x per live knot, plus a final
    # lane at the poly center whose powers are the poly features (the
    # 1/h^k scalings are folded into the weights host-side).  relu(u)^3 =
    # relu(u^3), and the mirrored (below-split) knots need relu(knot-x)^3
    # = -min(u^3, 0), so the sign fold goes into their weights and the
    # relu stage is one min over mirrored lanes + one max over direct
    # lanes (4x-mode tensor_scalar).  All bf16 (2x/4x DVE modes).
    nl = cfg.nlanes
    DD = pool.tile([128, nl, BSH], MM_DT, tag="DD", bufs=ib)
    nc.vector.tensor_tensor(
        DD[:], _bcast_mid(XB[:], nl), ICW[:], ALU.subtract
    )
    U2 = pool.tile([128, nl, BSH], MM_DT, tag="U2", bufs=ib)
    if cfg.u2_act:
        nc.scalar.activation(_flat(U2[:]), _flat(DD[:]), AF.Square)
    else:
        nc.vector.tensor_tensor(_flat(U2[:]), _flat(DD[:]), _flat(DD[:]),
                                ALU.mult)
    U3 = pool.tile([128, nl, BSH], MM_DT, tag="U3", bufs=ib)
    nc.vector.tensor_tensor(_flat(U3[:]), _flat(U2[:]), _flat(DD[:]), ALU.mult)
    R3 = pool.tile([128, ntap, BSH], MM_DT, tag="R3", bufs=ib)
    nc.vector.tensor_scalar(
        _flat(R3[:])[:, : nmir * BSH], _flat(U3[:])[:, : nmir * BSH],
        0.0, None, ALU.min,
    )
    nc.vector.tensor_scalar(
        _flat(R3[:])[:, nmir * BSH :],
        _flat(U3[:])[:, nmir * BSH : ntap * BSH], 0.0, None, ALU.max,
    )

    # out^T[o,b] = sum_k WT[:,k,:]^T @ rhs_k, K = ntiles*128
    PS = psum.tile([OUT_DIM, BSH], F32, tag="PS",
                   bufs=cfg.nbufs + (1 if cfg.skew else 0))
    rhss = [S[:], DD[:, ntap, :], U2[:, ntap, :], U3[:, ntap, :]]
    rhss += [R3[:, t, :] for t in range(ntap)]
    for k, rhs in enumerate(rhss):
        nc.tensor.matmul(
            PS[:], WT[:, k, :], rhs, start=(k == 0), stop=(k == len(rhss) - 1)
        )
    return PS


def build_program(
    cfg, iters: int = 1, pipelined: bool = False, loop_n: int = 1
):
    """One SPMD NeuronCore program; per-core inputs differ only in data.

    iters > 1 unrolls the full kernel back-to-back inside one NEFF, and
    loop_n > 1 wraps the unrolled body in a hardware For_i loop (total
    passes = iters * loop_n) - used to measure per-iteration HW execution
    time without a profiler while keeping the NEFF small.

    Successive passes write a small ring of output slices (a real stream
    writes each batch's result to a distinct buffer; reusing one address
    would add an artificial DRAM write-after-write serialization to the
    measurement).  Slice 0 always holds a complete pass result.
    """
    del pipelined  # legacy knob, superseded by the output ring
    nc = bass.Bass()
    xs = nc.declare_dram_parameter("xs", [IN_DIM, BSH], MM_DT, isOutput=False)
    # weights pre-transposed host-side to [i, k*o] so the load is one
    # contiguous-per-partition DMA
    wt = nc.declare_dram_parameter(
        "wt", [128, cfg.ntiles * OUT_DIM], MM_DT, isOutput=False
    )
    icw = nc.declare_dram_parameter(
        "icw", [128, cfg.nlanes * BSH], MM_DT, isOutput=False
    )
    bv = nc.declare_dram_parameter("bv", [OUT_DIM, 1], F32, isOutput=False)
    ring = min(iters, 8)
    outT = nc.declare_dram_parameter(
        "outT", [OUT_DIM, ring * BSH], F32, isOutput=True
    )

    with tile.TileContext(nc) as tc:
        with (
            tc.tile_pool(name="pool", bufs=1) as pool,
            tc.tile_pool(
                name="psum", bufs=1, space=bass.MemorySpace.PSUM,
            ) as psum,
        ):
            # loop-invariant constants, loaded once per NEFF execution:
            # tap offsets, output bias, w-shift, and the weight bank
            # (weights are pass-invariant, so they stay resident in SBUF)
            ICW = pool.tile([128, cfg.nlanes, BSH], MM_DT, tag="ICW", bufs=1)
            nc.sync.dma_start(_flat(ICW[:]), icw[:])
            BV = pool.tile([OUT_DIM, 1], F32, tag="BV", bufs=1)
            nc.sync.dma_start(BV[:], bv[:])
            WT = pool.tile([128, cfg.ntiles, OUT_DIM], MM_DT, tag="WT", bufs=1)
            nc.sync.dma_start(WT[:].rearrange("p a b -> p (a b)"), wt[:])

            def body():
                pending = None  # (PS, out-slice) awaiting its output stage
                for it in range(iters):
                    r = it % ring
                    o = outT[:, r * BSH : (r + 1) * BSH]
                    PS = _emit_iter(nc, pool, psum, xs, WT, ICW, cfg)
                    if not cfg.skew:
                        _emit_out(nc, pool, PS, o, BV, cfg)
                    else:
                        if pending is not None:
                            _emit_out(nc, pool, pending[0], pending[1], BV,
                                      cfg)
                        pending = (PS, o)
                if pending is not None:
                    _emit_out(nc, pool, pending[0], pending[1], BV, cfg)

            if loop_n > 1:
                with tc.For_i(0, loop_n, 1):
                    body()
            else:
                body()

    return nc


def _legalize_waits(nc):
    """Walrus codegen allows only one semaphore wait per compute/DMA
    instruction; move extra waits onto inserted same-engine NoOps."""
    for blk in nc.m.functions[0].blocks:
        out = []
        for ins in blk.instructions:
            si = ins.sync_info
            if si is not None and len(si.on_wait) > 1:
                waits = list(si.on_wait)
                for i, w in enumerate(waits[:-1]):
                    nop = mybir.InstNoOp(
                        name=f"{ins.name}-lw{i}", engine=ins.engine, ins=[], outs=[]
                    )
                    nop.sync_info = mybir.SyncInfo(on_wait=[w], on_update=[])
                    out.append(nop)
                ins.sync_info = mybir.SyncInfo(
                    on_wait=[waits[-1]], on_update=list(si.on_update)
                )
            out.append(ins)
        blk.instructions = out
    return nc


def prepare_inputs(x, grid, coef, scale_base, scale_sp, mask):
    x = np.ascontiguousarray(x, dtype=np.float32)
    grid = np.asarray(grid, dtype=np.float32)
    coef = np.asarray(coef, dtype=np.float64)
    g = grid[0].astype(np.float64)
    h = (g[-1] - g[0]) / (len(g) - 1)
    g0ext = g[0] - KDEG * h
    inv_h = 1.0 / h
    bias_v = -g0ext * inv_h

    vmin = float(x.min()) * inv_h + bias_v
    vmax = float(x.max()) * inv_h + bias_v
    cfg = Cfg(inv_h, bias_v, vmin, vmax)

    import ml_dtypes
    from math import comb

    bfq = lambda a: np.asarray(a, np.float32).astype(ml_dtypes.bfloat16)

    # fold Delta^4 (and the 1/6) into per-tap weights: W[s,n]
    W = np.zeros((SIZE, NT))
    for j in range(NB):
        for m in range(KDEG + 2):
            W[:, j + m] += coef[:, j] / 6.0 * ((-1) ** m) * comb(KDEG + 1, m)
    # cubic-polynomial fold of taps n <= msp, rebased at cc
    a = np.zeros((SIZE, 4))
    for n in range(0, cfg.msp + 1):
        t = cfg.cc - n
        a[:, 0] += W[:, n] * t**3
        a[:, 1] += W[:, n] * 3 * t**2
        a[:, 2] += W[:, n] * 3 * t
        a[:, 3] += W[:, n]

    sbm = np.asarray(scale_base, np.float64) * np.asarray(mask, np.float64)
    sspm = np.asarray(scale_sp, np.float64) * np.asarray(mask, np.float64)
    # 1/h^k folds for the x-space lane bank; mirrored knots get the
    # relu(knot-x)^3 = -min(u^3,0) sign fold
    rows = [sbm, sspm * a[:, 1] * inv_h, sspm * a[:, 2] * inv_h**2,
            sspm * a[:, 3] * inv_h**3]
    rows += [sspm * W[:, n] * inv_h**3 * (-1.0 if n <= cfg.msp else 1.0)
             for n in cfg.live]
    wt = np.empty((cfg.ntiles * 128, OUT_DIM), np.float32)
    for k, r in enumerate(rows):
        wt[k * 128 : (k + 1) * 128] = r.reshape(OUT_DIM, IN_DIM).T
    # [k*i, o] -> [i, k*o] so each partition's weights are contiguous
    wt = np.ascontiguousarray(
        wt.reshape(cfg.ntiles, IN_DIM, OUT_DIM).transpose(1, 0, 2).reshape(
            IN_DIM, cfg.ntiles * OUT_DIM
        )
    ).astype(mybir.dt.np(MM_DT))

    # per-o output bias: constant poly term summed over i
    bv = np.ascontiguousarray(
        (sspm * a[:, 0]).reshape(OUT_DIM, IN_DIM).sum(axis=1)[:, None],
        dtype=np.float32,
    )
    # lane offsets: knot x-positions, then the poly-center lane
    offs = bfq([g0ext + n * h for n in cfg.live] + [g0ext + cfg.cc * h])
    icw = np.ascontiguousarray(
        np.broadcast_to(
            np.repeat(offs, BSH)[None, :], (128, cfg.nlanes * BSH)
        )
    )

    xT = np.ascontiguousarray(x.T).astype(mybir.dt.np(MM_DT))  # [i, b] bf16
    in_maps = [
        {
            "xs": np.ascontiguousarray(xT[:, c * BSH : (c + 1) * BSH]),
            "wt": wt,
            "icw": icw,
            "bv": bv,
        }
        for c in range(N_CORES)
    ]
    return in_maps, cfg


class Runner:
    """AOT-compiled fast-dispatch executor for a Bass program on N cores.

    Compiles once (jit trace + NEFF build happen here, not per call);
    subsequent __call__s hit JAX's C++ fast path - per-call cost is the
    axon dispatch plus device execution only.
    """

    def __init__(self, nc, n_cores: int = N_CORES):
        import jax
        from jax.sharding import Mesh, NamedSharding, PartitionSpec

        from concourse import bass2jax
        from concourse.bass2jax import (
            _bass_exec_p,
            fast_dispatch_compile,
            install_neuronx_cc_hook,
        )

        try:
            from jax.experimental.shard_map import shard_map
        except ImportError:  # newer jax
            from jax import shard_map

        install_neuronx_cc_hook()
        self.jax = jax
        self.n_cores = n_cores
        part_name = nc.partition_id_tensor.name if nc.partition_id_tensor else None
        assert nc.dbg_addr is None

        in_names, in_shapes, out_names, out_avals = [], [], [], []
        for alloc in nc.m.functions[0].allocations:
            if not isinstance(alloc, mybir.MemoryLocationSet):
                continue
            name = alloc.memorylocations[0].name
            if alloc.kind == "ExternalInput":
                if name != part_name:
                    in_names.append(name)
                    in_shapes.append(
                        (tuple(alloc.tensor_shape), mybir.dt.np(alloc.dtype))
                    )
            elif alloc.kind == "ExternalOutput":
                out_names.append(name)
                out_avals.append(
                    jax.core.ShapedArray(
                        tuple(alloc.tensor_shape), mybir.dt.np(alloc.dtype)
                    )
                )
        self.in_names = in_names
        self.out_names = out_names
        self.out_avals = out_avals
        # The kernel writes every element of its outputs, so they are not
        # passed as (donated zero) operands - results are fresh buffers.
        all_in_names = list(in_names)
        if part_name is not None:
            all_in_names.append(part_name)

        def _body(*args):
            operands = list(args)
            if part_name is not None:
                operands.append(bass2jax.partition_id_tensor())
            outs = _bass_exec_p.bind(
                *operands,
                out_avals=tuple(out_avals),
                in_names=tuple(all_in_names),
                out_names=tuple(out_names),
                lowering_input_output_aliases=(),
                sim_require_finite=True,
                sim_require_nnan=True,
                nc=nc,
            )
            return tuple(outs)

        devices = jax.devices()[:n_cores]
        self.mesh = Mesh(np.asarray(devices), ("core",))
        self.sharding = NamedSharding(self.mesh, PartitionSpec("core"))
        in_specs = (PartitionSpec("core"),) * len(in_names)
        out_specs = (PartitionSpec("core"),) * len(out_names)
        jitted = jax.jit(
            shard_map(
                _body,
                mesh=self.mesh,
                in_specs=in_specs,
                out_specs=out_specs,
                check_rep=False,
            ),
            keep_unused=True,
        )

        def compile_fn():
            abstract = [
                jax.ShapeDtypeStruct((n_cores * s[0], *s[1:]), d)
                for (s, d) in in_shapes
            ]
            return jitted.lower(*abstract).compile()

        self.compiled = fast_dispatch_compile(compile_fn)

    def stage(self, in_maps):
        """Concat per-core inputs on axis 0 and put on device (committed)."""
        concat = [
            np.concatenate(
                [np.asarray(in_maps[c][nm]) for c in range(self.n_cores)], axis=0
            )
            for nm in self.in_names
        ]
        args = [self.jax.device_put(a, self.sharding) for a in concat]
        self.jax.block_until_ready(args)
        return args

    def __call__(self, args):
        return self.compiled(*args)

    def fetch_np(self, outs):
        """outs -> list of per-core np arrays for output 0."""
        arr = np.asarray(outs[0])
        s = self.out_avals[0].shape
        return arr.reshape(self.n_cores, *s)


def _assemble(per_core_outT):
    """per-core outT [OUT_DIM, BSH] -> full [BATCH, OUT_DIM]."""
    return np.ascontiguousarray(
        np.concatenate([o.T for o in per_core_outT], axis=0), dtype=np.float32
    )


def run(inputs: dict, trace: bool = False, **spmd_kwargs):
    """Stock-path execution (kept for debugging / fallback)."""
    from concourse.bass_utils import run_bass_kernel_spmd

    in_maps, cfg = prepare_inputs(**inputs)
    nc = _legalize_waits(build_program(cfg))
    res = run_bass_kernel_spmd(
        nc, in_maps, list(range(N_CORES)), trace=trace, **spmd_kwargs
    )
    out = _assemble([np.asarray(res.results[c]["outT"]) for c in range(N_CORES)])
    return out, res


def kernel(**inputs) -> np.ndarray:
    assert inputs["x"].shape == (BATCH, IN_DIM)
    in_maps, cfg = prepare_inputs(**inputs)
    nc = _legalize_waits(build_program(cfg))
    try:
        runner = Runner(nc)
        outs = runner(runner.stage(in_maps))
        return _assemble(list(runner.fetch_np(outs)))
    except Exception:
        from concourse.bass_utils import run_bass_kernel_spmd

        res = run_bass_kernel_spmd(nc, in_maps, list(range(N_CORES)))
        return _assemble(
            [np.asarray(res.results[c]["outT"]) for c in range(N_CORES)]
        )


# revision 35
# speedup vs baseline: 6.4994x; 1.0108x over previous
"""KANLayer (in=128, out=128, num=5, k=3, batch=1024) on 8 trn2 NeuronCores.

Math: out[b,o] = sum_i mask*scale_base*silu(x[b,i])
              + sum_i mask*scale_sp*sum_j coef[(o,i),j]*B_j(x[b,i])
The reference grid is a uniform linspace broadcast to all rows, so the
Cox-de-Boor bases are cardinal cubic B-splines, B_j(v) = Delta^4
relu(v-n)^3/6 at n=j with v = (x - g0ext)/h.  The Delta^4 is a fixed
linear map from truncated-power taps to bases, so it is folded into the
spline weights host-side: y_sp = sum_n W[s,n]*relu(v-n)^3 with
W = (coef/6) @ Delta4^T.  Three exact host-side rewrites then keep every
on-device lane value small enough for bf16 throughout:

  * taps with knot >= max(v) are identically zero -> dropped;
  * taps with knot <= min(v) satisfy relu(v-n)^3 = (v-n)^3 -> folded
    into a cubic polynomial;
  * live taps below the range midpoint use the mirror identity
    relu(v-n)^3 = (v-n)^3 + relu(n-v)^3, the cubic again folded into
    the polynomial.

Everything is evaluated in x-space (u = x - knot_x, the 1/h^k scales
folded into the weights).  The device-side work per pass is one bf16
lane bank [128, 7, 128] = {6 live knots, 1 poly-center lane} built by
four DVE ops (subtract; cube via square+multiply with the square on the
scalar engine; relu as min over mirrored lanes + max over direct lanes,
with the mirror sign folded into the weights since relu(knot-x)^3 =
-min(u^3, 0) and relu/cube commute), plus silu on the scalar engine.
The poly-center lane's powers in DD/U2/U3 are the polynomial features
w, w^2, w^3 for free, and the constant term rides the PSUM->SBUF output
copy as a per-partition bias.  The contraction is 10 accumulated
128x128x128 bf16 PE matmuls per core against a weight bank that stays
resident in SBUF.  All elementwise ops run in bf16 (2x/4x DVE modes).

Sharding: batch 1024 -> 128 per core (independent; no collectives).

Execution: the Bass program is AOT-compiled once into a PJRT executable
(fast-dispatch, no per-call retrace) and dispatched on cores 0-7; falls
back to the stock run_bass_kernel_spmd path on any failure.
"""

import numpy as np

import concourse.bass as bass
import concourse.mybir as mybir
import concourse.tile as tile

AF = mybir.ActivationFunctionType
ALU = mybir.AluOpType
F32 = mybir.dt.float32
BF16 = mybir.dt.bfloat16

N_CORES = 8
BATCH = 1024
IN_DIM = 128
OUT_DIM = 128
NUM, KDEG = 5, 3
NB = NUM + KDEG          # 8 basis functions
NT = NB + KDEG + 1       # 12 truncated-power taps
BSH = BATCH // N_CORES   # 128 batch elems per core
SIZE = IN_DIM * OUT_DIM

MM_DT = BF16  # matmul operand dtype


def _bcast_mid(ap2d, n):
    """[128, F] AP -> [128, n, F] with zero-stride middle dim."""
    p, f = ap2d.shape
    return ap2d.rearrange("p (a b) -> p a b", a=1).broadcast_to([p, n, f])


def _flat(ap3d):
    """[128, a, b] AP -> [128, a*b]."""
    return ap3d.rearrange("p a b -> p (a b)")


class Cfg:
    """Data-dependent program constants (live taps, split, poly center)."""

    def __init__(self, inv_h, bias_v, vmin, vmax):
        self.inv_h = float(inv_h)
        self.bias_v = float(bias_v)
        nlo = int(np.floor(vmin))          # taps <= nlo: always-on cubic
        nhi = int(np.ceil(vmax))           # taps >= nhi: identically zero
        self.cc = float((vmin + vmax) / 2)  # poly/tap rebase center
        self.msp = int(np.floor(self.cc))   # mirror split knot
        self.live = [n for n in range(max(nlo + 1, 0), min(nhi, NT))]
        self.nmir = sum(1 for n in self.live if n <= self.msp)
        self.ntap = len(self.live)
        self.nlanes = self.ntap + 1        # taps + the w-lane (t=0)
        self.ntiles = 4 + self.ntap        # silu, w, w^2, w^3, taps
        self.g0ext = 0.0                   # extended-grid origin (x-space)
        self.h = 1.0 / self.inv_h
        # engine assignment knobs (chosen by timeline-sim + on-device sweep)
        # (GPSIMD cannot access PSUM, so o_eng is scalar or vector only)
        self.o_eng = "split"   # PSUM->SBUF output copy engine
        self.u2_act = True     # lane square on Act engine (else DVE)
        self.nbufs = 3         # passes in flight (tile buffer depth)
        self.out_pool = False  # issue the output DMA from the Pool queue
        self.skew = True       # emit pass k's output stage after pass k+1


def _emit_out(nc, pool, PS, outT, BV, cfg):
    """Output stage: PSUM -> SBUF copy (+ poly-constant bias), then DMA."""
    O = pool.tile([OUT_DIM, BSH], F32, tag="O", bufs=cfg.nbufs)
    if cfg.o_eng == "vector":
        nc.vector.tensor_scalar(O[:], PS[:], BV[:, 0:1], None, ALU.add)
    elif cfg.o_eng == "split":  # halve the copy across DVE and Act
        h = BSH // 2
        nc.vector.tensor_scalar(O[:, :h], PS[:, :h], BV[:, 0:1], None, ALU.add)
        nc.scalar.activation(O[:, h:], PS[:, h:], AF.Identity, bias=BV[:, 0:1])
    else:
        nc.scalar.activation(O[:], PS[:], AF.Identity, bias=BV[:, 0:1])
    (nc.gpsimd if cfg.out_pool else nc.sync).dma_start(outT[:, :], O[:])


def _emit_iter(nc, pool, psum, xs, WT, ICW, cfg):
    """One pass's compute: load, lane bank, 4+ntap matmuls -> PSUM tile."""
    ntap, nmir = cfg.ntap, cfg.nmir
    ib = cfg.nbufs  # intermediate-tile buffering (passes in flight)
    # multi-buffered input load (x arrives pre-cast to bf16): later
    # passes' DMAs issue while this pass computes (standard prefetch)
    XB = pool.tile([128, BSH], MM_DT, tag="XB", bufs=ib)
    nc.sync.dma_start(XB[:], xs[:])

    S = pool.tile([128, BSH], MM_DT, tag="S", bufs=ib)  # silu(x), K-tile 0
    nc.scalar.activation(S[:], XB[:], AF.Silu)

    # lane bank in x-space: u = x - knot_x per live knot, plus a final
    # lane at the poly center whose powers are the poly features (the
    # 1/h^k scalings are folded into the weights host-side).  relu(u)^3 =
    # relu(u^3), and the mirrored (below-split) knots need relu(knot-x)^3
    # = -min(u^3, 0), so the sign fold goes into their weights and the
    # relu stage is one min over mirrored lanes + one max over direct
    # lanes (4x-mode tensor_scalar).  All bf16 (2x/4x DVE modes).
    nl = cfg.nlanes
    DD = pool.tile([128, nl, BSH], MM_DT, tag="DD", bufs=ib)
    nc.vector.tensor_tensor(
        DD[:], _bcast_mid(XB[:], nl), ICW[:], ALU.subtract
    )
    U2 = pool.tile([128, nl, BSH], MM_DT, tag="U2", bufs=ib)
    if cfg.u2_act:
        nc.scalar.activation(_flat(U2[:]), _flat(DD[:]), AF.Square)
    else:
        nc.vector.tensor_tensor(_flat(U2[:]), _flat(DD[:]), _flat(DD[:]),
                                ALU.mult)
    U3 = pool.tile([128, nl, BSH], MM_DT, tag="U3", bufs=ib)
    nc.vector.tensor_tensor(_flat(U3[:]), _flat(U2[:]), _flat(DD[:]), ALU.mult)
    R3 = pool.tile([128, ntap, BSH], MM_DT, tag="R3", bufs=ib)
    nc.vector.tensor_scalar(
        _flat(R3[:])[:, : nmir * BSH], _flat(U3[:])[:, : nmir * BSH],
        0.0, None, ALU.min,
    )
    nc.vector.tensor_scalar(
        _flat(R3[:])[:, nmir * BSH :],
        _flat(U3[:])[:, nmir * BSH : ntap * BSH], 0.0, None, ALU.max,
    )

    # out^T[o,b] = sum_k WT[:,k,:]^T @ rhs_k, K = ntiles*128
    PS = psum.tile([OUT_DIM, BSH], F32, tag="PS",
                   bufs=cfg.nbufs + (1 if cfg.skew else 0))
    rhss = [S[:], DD[:, ntap, :], U2[:, ntap, :], U3[:, ntap, :]]
    rhss += [R3[:, t, :] for t in range(ntap)]
    for k, rhs in enumerate(rhss):
        nc.tensor.matmul(
            PS[:], WT[:, k, :], rhs, start=(k == 0), stop=(k == len(rhss) - 1)
        )
    return PS


def build_program(
    cfg, iters: int = 1, pipelined: bool = False, loop_n: int = 1
):
    """One SPMD NeuronCore program; per-core inputs differ only in data.

    iters > 1 unrolls the full kernel back-to-back inside one NEFF, and
    loop_n > 1 wraps the unrolled body in a hardware For_i loop (total
    passes = iters * loop_n) - used to measure per-iteration HW execution
    time without a profiler while keeping the NEFF small.

    Successive passes write a small ring of output slices (a real stream
    writes each batch's result to a distinct buffer; reusing one address
    would add an artificial DRAM write-after-write serialization to the
    measurement).  Slice 0 always holds a complete pass result.
    """
    del pipelined  # legacy knob, superseded by the output ring
    nc = bass.Bass()
    xs = nc.declare_dram_parameter("xs", [IN_DIM, BSH], MM_DT, isOutput=False)
    # weights pre-transposed host-side to [i, k*o] so the load is one
    # contiguous-per-partition DMA
    wt = nc.declare_dram_parameter(
        "wt", [128, cfg.ntiles * OUT_DIM], MM_DT, isOutput=False
    )
    icw = nc.declare_dram_parameter(
        "icw", [128, cfg.nlanes * BSH], MM_DT, isOutput=False
    )
    bv = nc.declare_dram_parameter("bv", [OUT_DIM, 1], F32, isOutput=False)
    ring = min(iters, 8)
    outT = nc.declare_dram_parameter(
        "outT", [OUT_DIM, ring * BSH], F32, isOutput=True
    )

    with tile.TileContext(nc) as tc:
        with (
            tc.tile_pool(name="pool", bufs=1) as pool,
            tc.tile_pool(
                name="psum", bufs=1, space=bass.MemorySpace.PSUM,
            ) as psum,
        ):
            # loop-invariant constants, loaded once per NEFF execution:
            # tap offsets, output bias, w-shift, and the weight bank
            # (weights are pass-invariant, so they stay resident in SBUF)
            ICW = pool.tile([128, cfg.nlanes, BSH], MM_DT, tag="ICW", bufs=1)
            nc.sync.dma_start(_flat(ICW[:]), icw[:])
            BV = pool.tile([OUT_DIM, 1], F32, tag="BV", bufs=1)
            nc.sync.dma_start(BV[:], bv[:])
            WT = pool.tile([128, cfg.ntiles, OUT_DIM], MM_DT, tag="WT", bufs=1)
            nc.sync.dma_start(WT[:].rearrange("p a b -> p (a b)"), wt[:])

            def body():
                pending = None  # (PS, out-slice) awaiting its output stage
                for it in range(iters):
                    r = it % ring
                    o = outT[:, r * BSH : (r + 1) * BSH]
                    PS = _emit_iter(nc, pool, psum, xs, WT, ICW, cfg)
                    if not cfg.skew:
                        _emit_out(nc, pool, PS, o, BV, cfg)
                    else:
                        if pending is not None:
                            _emit_out(nc, pool, pending[0], pending[1], BV,
                                      cfg)
                        pending = (PS, o)
                if pending is not None:
                    _emit_out(nc, pool, pending[0], pending[1], BV, cfg)

            if loop_n > 1:
                with tc.For_i(0, loop_n, 1):
                    body()
            else:
                body()

    return nc


def _legalize_waits(nc):
    """Walrus codegen allows only one semaphore wait per compute/DMA
    instruction; move extra waits onto inserted same-engine NoOps."""
    for blk in nc.m.functions[0].blocks:
        out = []
        for ins in blk.instructions:
            si = ins.sync_info
            if si is not None and len(si.on_wait) > 1:
                waits = list(si.on_wait)
                for i, w in enumerate(waits[:-1]):
                    nop = mybir.InstNoOp(
                        name=f"{ins.name}-lw{i}", engine=ins.engine, ins=[], outs=[]
                    )
                    nop.sync_info = mybir.SyncInfo(on_wait=[w], on_update=[])
                    out.append(nop)
                ins.sync_info = mybir.SyncInfo(
                    on_wait=[waits[-1]], on_update=list(si.on_update)
                )
            out.append(ins)
        blk.instructions = out
    return nc


def prepare_inputs(x, grid, coef, scale_base, scale_sp, mask):
    x = np.ascontiguousarray(x, dtype=np.float32)
    grid = np.asarray(grid, dtype=np.float32)
    coef = np.asarray(coef, dtype=np.float64)
    g = grid[0].astype(np.float64)
    h = (g[-1] - g[0]) / (len(g) - 1)
    g0ext = g[0] - KDEG * h
    inv_h = 1.0 / h
    bias_v = -g0ext * inv_h

    vmin = float(x.min()) * inv_h + bias_v
    vmax = float(x.max()) * inv_h + bias_v
    cfg = Cfg(inv_h, bias_v, vmin, vmax)

    import ml_dtypes
    from math import comb

    bfq = lambda a: np.asarray(a, np.float32).astype(ml_dtypes.bfloat16)

    # fold Delta^4 (and the 1/6) into per-tap weights: W[s,n]
    W = np.zeros((SIZE, NT))
    for j in range(NB):
        for m in range(KDEG + 2):
            W[:, j + m] += coef[:, j] / 6.0 * ((-1) ** m) * comb(KDEG + 1, m)
    # cubic-polynomial fold of taps n <= msp, rebased at cc
    a = np.zeros((SIZE, 4))
    for n in range(0, cfg.msp + 1):
        t = cfg.cc - n
        a[:, 0] += W[:, n] * t**3
        a[:, 1] += W[:, n] * 3 * t**2
        a[:, 2] += W[:, n] * 3 * t
        a[:, 3] += W[:, n]

    sbm = np.asarray(scale_base, np.float64) * np.asarray(mask, np.float64)
    sspm = np.asarray(scale_sp, np.float64) * np.asarray(mask, np.float64)
    # 1/h^k folds for the x-space lane bank; mirrored knots get the
    # relu(knot-x)^3 = -min(u^3,0) sign fold
    rows = [sbm, sspm * a[:, 1] * inv_h, sspm * a[:, 2] * inv_h**2,
            sspm * a[:, 3] * inv_h**3]
    rows += [sspm * W[:, n] * inv_h**3 * (-1.0 if n <= cfg.msp else 1.0)
             for n in cfg.live]
    wt = np.empty((cfg.ntiles * 128, OUT_DIM), np.float32)
    for k, r in enumerate(rows):
        wt[k * 128 : (k + 1) * 128] = r.reshape(OUT_DIM, IN_DIM).T
    # [k*i, o] -> [i, k*o] so each partition's weights are contiguous
    wt = np.ascontiguousarray(
        wt.reshape(cfg.ntiles, IN_DIM, OUT_DIM).transpose(1, 0, 2).reshape(
            IN_DIM, cfg.ntiles * OUT_DIM
        )
    ).astype(mybir.dt.np(MM_DT))

    # per-o output bias: constant poly term summed over i
    bv = np.ascontiguousarray(
        (sspm * a[:, 0]).reshape(OUT_DIM, IN_DIM).sum(axis=1)[:, None],
        dtype=np.float32,
    )
    # lane offsets: knot x-positions, then the poly-center lane
    offs = bfq([g0ext + n * h for n in cfg.live] + [g0ext + cfg.cc * h])
    icw = np.ascontiguousarray(
        np.broadcast_to(
            np.repeat(offs, BSH)[None, :], (128, cfg.nlanes * BSH)
        )
    )

    xT = np.ascontiguousarray(x.T).astype(mybir.dt.np(MM_DT))  # [i, b] bf16
    in_maps = [
        {
            "xs": np.ascontiguousarray(xT[:, c * BSH : (c + 1) * BSH]),
            "wt": wt,
            "icw": icw,
            "bv": bv,
        }
        for c in range(N_CORES)
    ]
    return in_maps, cfg


class Runner:
    """AOT-compiled fast-dispatch executor for a Bass program on N cores.

    Compiles once (jit trace + NEFF build happen here, not per call);
    subsequent __call__s hit JAX's C++ fast path - per-call cost is the
    axon dispatch plus device execution only.
    """

    def __init__(self, nc, n_cores: int = N_CORES):
        import jax
        from jax.sharding import Mesh, NamedSharding, PartitionSpec

        from concourse import bass2jax
        from concourse.bass2jax import (
            _bass_exec_p,
            fast_dispatch_compile,
            install_neuronx_cc_hook,
        )

        try:
            from jax.experimental.shard_map import shard_map
        except ImportError:  # newer jax
            from jax import shard_map

        install_neuronx_cc_hook()
        self.jax = jax
        self.n_cores = n_cores
        part_name = nc.partition_id_tensor.name if nc.partition_id_tensor else None
        assert nc.dbg_addr is None

        in_names, in_shapes, out_names, out_avals = [], [], [], []
        for alloc in nc.m.functions[0].allocations:
            if not isinstance(alloc, mybir.MemoryLocationSet):
                continue
            name = alloc.memorylocations[0].name
            if alloc.kind == "ExternalInput":
                if name != part_name:
                    in_names.append(name)
                    in_shapes.append(
                        (tuple(alloc.tensor_shape), mybir.dt.np(alloc.dtype))
                    )
            elif alloc.kind == "ExternalOutput":
                out_names.append(name)
                out_avals.append(
                    jax.core.ShapedArray(
                        tuple(alloc.tensor_shape), mybir.dt.np(alloc.dtype)
                    )
                )
        self.in_names = in_names
        self.out_names = out_names
        self.out_avals = out_avals
        # The kernel writes every element of its outputs, so they are not
        # passed as (donated zero) operands - results are fresh buffers.
        all_in_names = list(in_names)
        if part_name is not None:
            all_in_names.append(part_name)

        def _body(*args):
            operands = list(args)
            if part_name is not None:
                operands.append(bass2jax.partition_id_tensor())
            outs = _bass_exec_p.bind(
                *operands,
                out_avals=tuple(out_avals),
                in_names=tuple(all_in_names),
                out_names=tuple(out_names),
                lowering_input_output_aliases=(),
                sim_require_finite=True,
                sim_require_nnan=True,
                nc=nc,
            )
            return tuple(outs)

        devices = jax.devices()[:n_cores]
        self.mesh = Mesh(np.asarray(devices), ("core",))
        self.sharding = NamedSharding(self.mesh, PartitionSpec("core"))
        in_specs = (PartitionSpec("core"),) * len(in_names)
        out_specs = (PartitionSpec("core"),) * len(out_names)
        jitted = jax.jit(
            shard_map(
                _body,
                mesh=self.mesh,
                in_specs=in_specs,
                out_specs=out_specs,
                check_rep=False,
            ),
            keep_unused=True,
        )

        def compile_fn():
            abstract = [
                jax.ShapeDtypeStruct((n_cores * s[0], *s[1:]), d)
                for (s, d) in in_shapes
            ]
            return jitted.lower(*abstract).compile()

        self.compiled = fast_dispatch_compile(compile_fn)

    def stage(self, in_maps):
        """Concat per-core inputs on axis 0 and put on device (committed)."""
        concat = [
            np.concatenate(
                [np.asarray(in_maps[c][nm]) for c in range(self.n_cores)], axis=0
            )
            for nm in self.in_names
        ]
        args = [self.jax.device_put(a, self.sharding) for a in concat]
        self.jax.block_until_ready(args)
        return args

    def __call__(self, args):
        return self.compiled(*args)

    def fetch_np(self, outs):
        """outs -> list of per-core np arrays for output 0."""
        arr = np.asarray(outs[0])
        s = self.out_avals[0].shape
        return arr.reshape(self.n_cores, *s)


def _assemble(per_core_outT):
    """per-core outT [OUT_DIM, BSH] -> full [BATCH, OUT_DIM]."""
    return np.ascontiguousarray(
        np.concatenate([o.T for o in per_core_outT], axis=0), dtype=np.float32
    )


def run(inputs: dict, trace: bool = False, **spmd_kwargs):
    """Stock-path execution (kept for debugging / fallback)."""
    from concourse.bass_utils import run_bass_kernel_spmd

    in_maps, cfg = prepare_inputs(**inputs)
    nc = _legalize_waits(build_program(cfg))
    res = run_bass_kernel_spmd(
        nc, in_maps, list(range(N_CORES)), trace=trace, **spmd_kwargs
    )
    out = _assemble([np.asarray(res.results[c]["outT"]) for c in range(N_CORES)])
    return out, res


def kernel(**inputs) -> np.ndarray:
    assert inputs["x"].shape == (BATCH, IN_DIM)
    in_maps, cfg = prepare_inputs(**inputs)
    nc = _legalize_waits(build_program(cfg))
    try:
        runner = Runner(nc)
        outs = runner(runner.stage(in_maps))
        return _assemble(list(runner.fetch_np(outs)))
    except Exception:
        from concourse.bass_utils import run_bass_kernel_spmd

        res = run_bass_kernel_spmd(nc, in_maps, list(range(N_CORES)))
        return _assemble(
            [np.asarray(res.results[c]["outT"]) for c in range(N_CORES)]
        )


# revision 44
# speedup vs baseline: 6.5068x; 1.0011x over previous
"""KANLayer (in=128, out=128, num=5, k=3, batch=1024) on 8 trn2 NeuronCores.

Math: out[b,o] = sum_i mask*scale_base*silu(x[b,i])
              + sum_i mask*scale_sp*sum_j coef[(o,i),j]*B_j(x[b,i])
The reference grid is a uniform linspace broadcast to all rows, so the
Cox-de-Boor bases are cardinal cubic B-splines, B_j(v) = Delta^4
relu(v-n)^3/6 at n=j with v = (x - g0ext)/h.  The Delta^4 is a fixed
linear map from truncated-power taps to bases, so it is folded into the
spline weights host-side: y_sp = sum_n W[s,n]*relu(v-n)^3 with
W = (coef/6) @ Delta4^T.  Three exact host-side rewrites then keep every
on-device lane value small enough for bf16 throughout:

  * taps with knot >= max(v) are identically zero -> dropped;
  * taps with knot <= min(v) satisfy relu(v-n)^3 = (v-n)^3 -> folded
    into a cubic polynomial;
  * live taps below the range midpoint use the mirror identity
    relu(v-n)^3 = (v-n)^3 + relu(n-v)^3, the cubic again folded into
    the polynomial.

Everything is evaluated in x-space (u = x - knot_x, the 1/h^k scales
folded into the weights).  The device-side work per pass is one bf16
lane bank [128, 7, 128] = {6 live knots, 1 poly-center lane} built by
four DVE ops (subtract; cube via square+multiply with the square on the
scalar engine; relu as min over mirrored lanes + max over direct lanes,
with the mirror sign folded into the weights since relu(knot-x)^3 =
-min(u^3, 0) and relu/cube commute), plus silu on the scalar engine.
The poly-center lane's powers in DD/U2/U3 are the polynomial features
w, w^2, w^3 for free, and the constant term rides the PSUM->SBUF output
copy as a per-partition bias.  The contraction is 10 accumulated
128x128x128 bf16 PE matmuls per core against a weight bank that stays
resident in SBUF.  All elementwise ops run in bf16 (2x/4x DVE modes).

Sharding: batch 1024 -> 128 per core (independent; no collectives).

Execution: the Bass program is AOT-compiled once into a PJRT executable
(fast-dispatch, no per-call retrace) and dispatched on cores 0-7; falls
back to the stock run_bass_kernel_spmd path on any failure.
"""

import numpy as np

import concourse.bass as bass
import concourse.mybir as mybir
import concourse.tile as tile

AF = mybir.ActivationFunctionType
ALU = mybir.AluOpType
F32 = mybir.dt.float32
BF16 = mybir.dt.bfloat16

N_CORES = 8
BATCH = 1024
IN_DIM = 128
OUT_DIM = 128
NUM, KDEG = 5, 3
NB = NUM + KDEG          # 8 basis functions
NT = NB + KDEG + 1       # 12 truncated-power taps
BSH = BATCH // N_CORES   # 128 batch elems per core
SIZE = IN_DIM * OUT_DIM

MM_DT = BF16  # matmul operand dtype


def _bcast_mid(ap2d, n):
    """[128, F] AP -> [128, n, F] with zero-stride middle dim."""
    p, f = ap2d.shape
    return ap2d.rearrange("p (a b) -> p a b", a=1).broadcast_to([p, n, f])


def _flat(ap3d):
    """[128, a, b] AP -> [128, a*b]."""
    return ap3d.rearrange("p a b -> p (a b)")


class Cfg:
    """Data-dependent program constants (live taps, split, poly center)."""

    def __init__(self, inv_h, bias_v, vmin, vmax):
        self.inv_h = float(inv_h)
        self.bias_v = float(bias_v)
        nlo = int(np.floor(vmin))          # taps <= nlo: always-on cubic
        nhi = int(np.ceil(vmax))           # taps >= nhi: identically zero
        self.cc = float((vmin + vmax) / 2)  # poly/tap rebase center
        self.msp = int(np.floor(self.cc))   # mirror split knot
        self.live = [n for n in range(max(nlo + 1, 0), min(nhi, NT))]
        self.nmir = sum(1 for n in self.live if n <= self.msp)
        self.ntap = len(self.live)
        self.nlanes = self.ntap + 1        # taps + the w-lane (t=0)
        self.ntiles = 4 + self.ntap        # silu, w, w^2, w^3, taps
        self.h = 1.0 / self.inv_h
        self.g0ext = -self.bias_v * self.h  # extended-grid origin (x-space)
        self.cx = self.g0ext + self.cc * self.h  # poly center, x-space
        # engine assignment knobs (chosen by timeline-sim + on-device sweep)
        # (GPSIMD cannot access PSUM, so o_eng is scalar or vector only)
        self.o_eng = "split"   # PSUM->SBUF output copy engine
        self.u2_act = True     # lane square on Act engine (else DVE)
        self.nbufs = 3         # passes in flight (tile buffer depth)
        self.out_pool = False  # issue the output DMA from the Pool queue
        self.skew = True       # emit pass k's output stage after pass k+1
        # GpSimd per-op overhead is ~1us on real HW (cost model is wrong
        # about it) - never assign per-pass ops there
        self.poly_pool = False  # poly-feature lane pipeline on GpSimd
        self.o_dve_cols = 32   # columns of the output copy done on DVE
        self.dd2 = False       # two-op DD (mir lanes negated) + single max


def _emit_out(nc, pool, PS, outT, BV, cfg):
    """Output stage: PSUM -> SBUF copy (+ poly-constant bias), then DMA."""
    O = pool.tile([OUT_DIM, BSH], F32, tag="O", bufs=cfg.nbufs)
    if cfg.o_eng == "vector":
        nc.vector.tensor_scalar(O[:], PS[:], BV[:, 0:1], None, ALU.add)
    elif cfg.o_eng == "split":  # split the copy across DVE and Act
        h = cfg.o_dve_cols
        nc.vector.tensor_scalar(O[:, :h], PS[:, :h], BV[:, 0:1], None, ALU.add)
        nc.scalar.activation(O[:, h:], PS[:, h:], AF.Identity, bias=BV[:, 0:1])
    else:
        nc.scalar.activation(O[:], PS[:], AF.Identity, bias=BV[:, 0:1])
    (nc.gpsimd if cfg.out_pool else nc.sync).dma_start(outT[:, :], O[:])


def _emit_iter(nc, pool, psum, xs, WT, ICW, cfg):
    """One pass's compute: load, lane bank, 4+ntap matmuls -> PSUM tile."""
    ntap, nmir = cfg.ntap, cfg.nmir
    ib = cfg.nbufs  # intermediate-tile buffering (passes in flight)
    # multi-buffered input load (x arrives pre-cast to bf16): later
    # passes' DMAs issue while this pass computes (standard prefetch)
    XB = pool.tile([128, BSH], MM_DT, tag="XB", bufs=ib)
    nc.sync.dma_start(XB[:], xs[:])

    S = pool.tile([128, BSH], MM_DT, tag="S", bufs=ib)  # silu(x), K-tile 0
    nc.scalar.activation(S[:], XB[:], AF.Silu)

    # lane bank in x-space: u = x - knot_x per live knot, plus a final
    # lane at the poly center whose powers are the poly features (the
    # 1/h^k scalings are folded into the weights host-side).  relu(u)^3 =
    # relu(u^3), and the mirrored (below-split) knots need relu(knot-x)^3
    # = -min(u^3, 0), so the sign fold goes into their weights and the
    # relu stage is one min over mirrored lanes + one max over direct
    # lanes (4x-mode tensor_scalar).  All bf16 (2x/4x DVE modes).
    # with poly_pool the bank holds only the tap lanes; the poly-center
    # lane (no relu needed) runs as its own 3-op pipeline on GpSimd
    nb = ntap if cfg.poly_pool else cfg.nlanes
    DD = pool.tile([128, nb, BSH], MM_DT, tag="DD", bufs=ib)
    if cfg.dd2:
        # mirrored lanes hold knot-x directly, so the relu stage is one
        # max over all tap lanes and no weight sign fold is needed
        nc.vector.tensor_tensor(
            DD[:, :nmir, :], ICW[:, :nmir, :], _bcast_mid(XB[:], nmir),
            ALU.subtract,
        )
        nc.vector.tensor_tensor(
            DD[:, nmir:, :], _bcast_mid(XB[:], nb - nmir), ICW[:, nmir:nb, :],
            ALU.subtract,
        )
    else:
        nc.vector.tensor_tensor(
            DD[:], _bcast_mid(XB[:], nb), ICW[:, :nb, :], ALU.subtract
        )
    U2 = pool.tile([128, nb, BSH], MM_DT, tag="U2", bufs=ib)
    if cfg.u2_act:
        nc.scalar.activation(_flat(U2[:]), _flat(DD[:]), AF.Square)
    else:
        nc.vector.tensor_tensor(_flat(U2[:]), _flat(DD[:]), _flat(DD[:]),
                                ALU.mult)
    U3 = pool.tile([128, nb, BSH], MM_DT, tag="U3", bufs=ib)
    nc.vector.tensor_tensor(_flat(U3[:]), _flat(U2[:]), _flat(DD[:]), ALU.mult)
    R3 = pool.tile([128, ntap, BSH], MM_DT, tag="R3", bufs=ib)
    if cfg.dd2:
        nc.vector.tensor_scalar(
            _flat(R3[:]), _flat(U3[:])[:, : ntap * BSH], 0.0, None, ALU.max
        )
    else:
        nc.vector.tensor_scalar(
            _flat(R3[:])[:, : nmir * BSH], _flat(U3[:])[:, : nmir * BSH],
            0.0, None, ALU.min,
        )
        nc.vector.tensor_scalar(
            _flat(R3[:])[:, nmir * BSH :],
            _flat(U3[:])[:, nmir * BSH : ntap * BSH], 0.0, None, ALU.max,
        )
    if cfg.poly_pool:
        DP = pool.tile([128, BSH], MM_DT, tag="DP", bufs=ib)
        nc.gpsimd.tensor_scalar(DP[:], XB[:], cfg.cx, None, ALU.subtract)
        P2 = pool.tile([128, BSH], MM_DT, tag="P2", bufs=ib)
        nc.gpsimd.tensor_tensor(P2[:], DP[:], DP[:], ALU.mult)
        P3 = pool.tile([128, BSH], MM_DT, tag="P3", bufs=ib)
        nc.gpsimd.tensor_tensor(P3[:], P2[:], DP[:], ALU.mult)
        poly = [DP[:], P2[:], P3[:]]
    else:
        poly = [DD[:, ntap, :], U2[:, ntap, :], U3[:, ntap, :]]

    # out^T[o,b] = sum_k WT[:,k,:]^T @ rhs_k, K = ntiles*128
    PS = psum.tile([OUT_DIM, BSH], F32, tag="PS",
                   bufs=cfg.nbufs + (1 if cfg.skew else 0))
    rhss = [S[:]] + poly
    rhss += [R3[:, t, :] for t in range(ntap)]
    for k, rhs in enumerate(rhss):
        nc.tensor.matmul(
            PS[:], WT[:, k, :], rhs, start=(k == 0), stop=(k == len(rhss) - 1)
        )
    return PS


def build_program(
    cfg, iters: int = 1, pipelined: bool = False, loop_n: int = 1
):
    """One SPMD NeuronCore program; per-core inputs differ only in data.

    iters > 1 unrolls the full kernel back-to-back inside one NEFF, and
    loop_n > 1 wraps the unrolled body in a hardware For_i loop (total
    passes = iters * loop_n) - used to measure per-iteration HW execution
    time without a profiler while keeping the NEFF small.

    Successive passes write a small ring of output slices (a real stream
    writes each batch's result to a distinct buffer; reusing one address
    would add an artificial DRAM write-after-write serialization to the
    measurement).  Slice 0 always holds a complete pass result.
    """
    del pipelined  # legacy knob, superseded by the output ring
    nc = bass.Bass()
    xs = nc.declare_dram_parameter("xs", [IN_DIM, BSH], MM_DT, isOutput=False)
    # weights pre-transposed host-side to [i, k*o] so the load is one
    # contiguous-per-partition DMA
    wt = nc.declare_dram_parameter(
        "wt", [128, cfg.ntiles * OUT_DIM], MM_DT, isOutput=False
    )
    icw = nc.declare_dram_parameter(
        "icw", [128, cfg.nlanes * BSH], MM_DT, isOutput=False
    )
    bv = nc.declare_dram_parameter("bv", [OUT_DIM, 1], F32, isOutput=False)
    ring = min(iters, 8)
    outT = nc.declare_dram_parameter(
        "outT", [OUT_DIM, ring * BSH], F32, isOutput=True
    )

    with tile.TileContext(nc) as tc:
        with (
            tc.tile_pool(name="pool", bufs=1) as pool,
            tc.tile_pool(
                name="psum", bufs=1, space=bass.MemorySpace.PSUM,
            ) as psum,
        ):
            # loop-invariant constants, loaded once per NEFF execution:
            # tap offsets, output bias, w-shift, and the weight bank
            # (weights are pass-invariant, so they stay resident in SBUF)
            ICW = pool.tile([128, cfg.nlanes, BSH], MM_DT, tag="ICW", bufs=1)
            nc.sync.dma_start(_flat(ICW[:]), icw[:])
            BV = pool.tile([OUT_DIM, 1], F32, tag="BV", bufs=1)
            nc.sync.dma_start(BV[:], bv[:])
            WT = pool.tile([128, cfg.ntiles, OUT_DIM], MM_DT, tag="WT", bufs=1)
            nc.sync.dma_start(WT[:].rearrange("p a b -> p (a b)"), wt[:])

            def body():
                pending = None  # (PS, out-slice) awaiting its output stage
                for it in range(iters):
                    r = it % ring
                    o = outT[:, r * BSH : (r + 1) * BSH]
                    PS = _emit_iter(nc, pool, psum, xs, WT, ICW, cfg)
                    if not cfg.skew:
                        _emit_out(nc, pool, PS, o, BV, cfg)
                    else:
                        if pending is not None:
                            _emit_out(nc, pool, pending[0], pending[1], BV,
                                      cfg)
                        pending = (PS, o)
                if pending is not None:
                    _emit_out(nc, pool, pending[0], pending[1], BV, cfg)

            if loop_n > 1:
                with tc.For_i(0, loop_n, 1):
                    body()
            else:
                body()

    return nc


def _legalize_waits(nc):
    """Walrus codegen allows only one semaphore wait per compute/DMA
    instruction; move extra waits onto inserted same-engine NoOps."""
    for blk in nc.m.functions[0].blocks:
        out = []
        for ins in blk.instructions:
            si = ins.sync_info
            if si is not None and len(si.on_wait) > 1:
                waits = list(si.on_wait)
                for i, w in enumerate(waits[:-1]):
                    nop = mybir.InstNoOp(
                        name=f"{ins.name}-lw{i}", engine=ins.engine, ins=[], outs=[]
                    )
                    nop.sync_info = mybir.SyncInfo(on_wait=[w], on_update=[])
                    out.append(nop)
                ins.sync_info = mybir.SyncInfo(
                    on_wait=[waits[-1]], on_update=list(si.on_update)
                )
            out.append(ins)
        blk.instructions = out
    return nc


def prepare_inputs(x, grid, coef, scale_base, scale_sp, mask):
    x = np.ascontiguousarray(x, dtype=np.float32)
    grid = np.asarray(grid, dtype=np.float32)
    coef = np.asarray(coef, dtype=np.float64)
    g = grid[0].astype(np.float64)
    h = (g[-1] - g[0]) / (len(g) - 1)
    g0ext = g[0] - KDEG * h
    inv_h = 1.0 / h
    bias_v = -g0ext * inv_h

    vmin = float(x.min()) * inv_h + bias_v
    vmax = float(x.max()) * inv_h + bias_v
    cfg = Cfg(inv_h, bias_v, vmin, vmax)

    import ml_dtypes
    from math import comb

    bfq = lambda a: np.asarray(a, np.float32).astype(ml_dtypes.bfloat16)

    # fold Delta^4 (and the 1/6) into per-tap weights: W[s,n]
    W = np.zeros((SIZE, NT))
    for j in range(NB):
        for m in range(KDEG + 2):
            W[:, j + m] += coef[:, j] / 6.0 * ((-1) ** m) * comb(KDEG + 1, m)
    # cubic-polynomial fold of taps n <= msp, rebased at cc
    a = np.zeros((SIZE, 4))
    for n in range(0, cfg.msp + 1):
        t = cfg.cc - n
        a[:, 0] += W[:, n] * t**3
        a[:, 1] += W[:, n] * 3 * t**2
        a[:, 2] += W[:, n] * 3 * t
        a[:, 3] += W[:, n]

    sbm = np.asarray(scale_base, np.float64) * np.asarray(mask, np.float64)
    sspm = np.asarray(scale_sp, np.float64) * np.asarray(mask, np.float64)
    # 1/h^k folds for the x-space lane bank; mirrored knots get the
    # relu(knot-x)^3 = -min(u^3,0) sign fold
    rows = [sbm, sspm * a[:, 1] * inv_h, sspm * a[:, 2] * inv_h**2,
            sspm * a[:, 3] * inv_h**3]
    rows += [
        sspm * W[:, n] * inv_h**3
        * (-1.0 if (n <= cfg.msp and not cfg.dd2) else 1.0)
        for n in cfg.live
    ]
    wt = np.empty((cfg.ntiles * 128, OUT_DIM), np.float32)
    for k, r in enumerate(rows):
        wt[k * 128 : (k + 1) * 128] = r.reshape(OUT_DIM, IN_DIM).T
    # [k*i, o] -> [i, k*o] so each partition's weights are contiguous
    wt = np.ascontiguousarray(
        wt.reshape(cfg.ntiles, IN_DIM, OUT_DIM).transpose(1, 0, 2).reshape(
            IN_DIM, cfg.ntiles * OUT_DIM
        )
    ).astype(mybir.dt.np(MM_DT))

    # per-o output bias: constant poly term summed over i
    bv = np.ascontiguousarray(
        (sspm * a[:, 0]).reshape(OUT_DIM, IN_DIM).sum(axis=1)[:, None],
        dtype=np.float32,
    )
    # lane offsets: knot x-positions, then the poly-center lane
    offs = bfq([g0ext + n * h for n in cfg.live] + [g0ext + cfg.cc * h])
    icw = np.ascontiguousarray(
        np.broadcast_to(
            np.repeat(offs, BSH)[None, :], (128, cfg.nlanes * BSH)
        )
    )

    xT = np.ascontiguousarray(x.T).astype(mybir.dt.np(MM_DT))  # [i, b] bf16
    in_maps = [
        {
            "xs": np.ascontiguousarray(xT[:, c * BSH : (c + 1) * BSH]),
            "wt": wt,
            "icw": icw,
            "bv": bv,
        }
        for c in range(N_CORES)
    ]
    return in_maps, cfg


class Runner:
    """AOT-compiled fast-dispatch executor for a Bass program on N cores.

    Compiles once (jit trace + NEFF build happen here, not per call);
    subsequent __call__s hit JAX's C++ fast path - per-call cost is the
    axon dispatch plus device execution only.
    """

    def __init__(self, nc, n_cores: int = N_CORES):
        import jax
        from jax.sharding import Mesh, NamedSharding, PartitionSpec

        from concourse import bass2jax
        from concourse.bass2jax import (
            _bass_exec_p,
            fast_dispatch_compile,
            install_neuronx_cc_hook,
        )

        try:
            from jax.experimental.shard_map import shard_map
        except ImportError:  # newer jax
            from jax import shard_map

        install_neuronx_cc_hook()
        self.jax = jax
        self.n_cores = n_cores
        part_name = nc.partition_id_tensor.name if nc.partition_id_tensor else None
        assert nc.dbg_addr is None

        in_names, in_shapes, out_names, out_avals = [], [], [], []
        for alloc in nc.m.functions[0].allocations:
            if not isinstance(alloc, mybir.MemoryLocationSet):
                continue
            name = alloc.memorylocations[0].name
            if alloc.kind == "ExternalInput":
                if name != part_name:
                    in_names.append(name)
                    in_shapes.append(
                        (tuple(alloc.tensor_shape), mybir.dt.np(alloc.dtype))
                    )
            elif alloc.kind == "ExternalOutput":
                out_names.append(name)
                out_avals.append(
                    jax.core.ShapedArray(
                        tuple(alloc.tensor_shape), mybir.dt.np(alloc.dtype)
                    )
                )
        self.in_names = in_names
        self.out_names = out_names
        self.out_avals = out_avals
        # The kernel writes every element of its outputs, so they are not
        # passed as (donated zero) operands - results are fresh buffers.
        all_in_names = list(in_names)
        if part_name is not None:
            all_in_names.append(part_name)

        def _body(*args):
            operands = list(args)
            if part_name is not None:
                operands.append(bass2jax.partition_id_tensor())
            outs = _bass_exec_p.bind(
                *operands,
                out_avals=tuple(out_avals),
                in_names=tuple(all_in_names),
                out_names=tuple(out_names),
                lowering_input_output_aliases=(),
                sim_require_finite=True,
                sim_require_nnan=True,
                nc=nc,
            )
            return tuple(outs)

        devices = jax.devices()[:n_cores]
        self.mesh = Mesh(np.asarray(devices), ("core",))
        self.sharding = NamedSharding(self.mesh, PartitionSpec("core"))
        in_specs = (PartitionSpec("core"),) * len(in_names)
        out_specs = (PartitionSpec("core"),) * len(out_names)
        jitted = jax.jit(
            shard_map(
                _body,
                mesh=self.mesh,
                in_specs=in_specs,
                out_specs=out_specs,
                check_rep=False,
            ),
            keep_unused=True,
        )

        def compile_fn():
            abstract = [
                jax.ShapeDtypeStruct((n_cores * s[0], *s[1:]), d)
                for (s, d) in in_shapes
            ]
            return jitted.lower(*abstract).compile()

        self.compiled = fast_dispatch_compile(compile_fn)

    def stage(self, in_maps):
        """Concat per-core inputs on axis 0 and put on device (committed)."""
        concat = [
            np.concatenate(
                [np.asarray(in_maps[c][nm]) for c in range(self.n_cores)], axis=0
            )
            for nm in self.in_names
        ]
        args = [self.jax.device_put(a, self.sharding) for a in concat]
        self.jax.block_until_ready(args)
        return args

    def __call__(self, args):
        return self.compiled(*args)

    def fetch_np(self, outs):
        """outs -> list of per-core np arrays for output 0."""
        arr = np.asarray(outs[0])
        s = self.out_avals[0].shape
        return arr.reshape(self.n_cores, *s)


def _assemble(per_core_outT):
    """per-core outT [OUT_DIM, BSH] -> full [BATCH, OUT_DIM]."""
    return np.ascontiguousarray(
        np.concatenate([o.T for o in per_core_outT], axis=0), dtype=np.float32
    )


def run(inputs: dict, trace: bool = False, **spmd_kwargs):
    """Stock-path execution (kept for debugging / fallback)."""
    from concourse.bass_utils import run_bass_kernel_spmd

    in_maps, cfg = prepare_inputs(**inputs)
    nc = _legalize_waits(build_program(cfg))
    res = run_bass_kernel_spmd(
        nc, in_maps, list(range(N_CORES)), trace=trace, **spmd_kwargs
    )
    out = _assemble([np.asarray(res.results[c]["outT"]) for c in range(N_CORES)])
    return out, res


def kernel(**inputs) -> np.ndarray:
    assert inputs["x"].shape == (BATCH, IN_DIM)
    in_maps, cfg = prepare_inputs(**inputs)
    nc = _legalize_waits(build_program(cfg))
    try:
        runner = Runner(nc)
        outs = runner(runner.stage(in_maps))
        return _assemble(list(runner.fetch_np(outs)))
    except Exception:
        from concourse.bass_utils import run_bass_kernel_spmd

        res = run_bass_kernel_spmd(nc, in_maps, list(range(N_CORES)))
        return _assemble(
            [np.asarray(res.results[c]["outT"]) for c in range(N_CORES)]
        )
